# revision 1
# baseline (speedup 1.0000x reference)
"""nn_ConvModel — Bass/Tile kernel, data-parallel over 8 TRN2 NeuronCores.

Strategy (per sharding_hint): batch dim of `image` sharded 8 ways, tiny
3-bit-quantized weights replicated; the two data-dependent activation
quant scales (s1 for lin, s3 for the depthwise-conv output) are computed
on-device as shard-local abs-maxes + AllReduce(max).  The input scale s0
(pure function of the input) and the final logits scale s5 (160 KB
tensor) are applied on the host, exactly.

Device layout (per core, batch shard b=512):
  partitions = (l%4)*32 + channel%32   [l = sequence pos 0..27, 28=7*4]
  free       = batch
  * stage-B linear:  lhsT[(f28,lp4)+ones=113, (l4,c32)=128] block-diag in
    l with the bias row b1/k1 folded in; one matmul per (channel-group g
    of 12, l-slab s of 7), N=512.
  * depthwise conv:  block-Toeplitz 128x128 weights W_d (d=-2..2), ~29
    accumulated TensorE matmuls per (g,s); no transposes anywhere.
  * final linear:    Wf rearranged to the same (l4,c32) partition order,
    84 accumulating matmuls into one [10,512] PSUM tile.
  * fake-quant rounding = (+1.5*2^23, -1.5*2^23) round-to-nearest-even,
    spread across ScalarE (affine+M fused into Identity-activation),
    VectorE and GpSimd; tanh on ScalarE with device scale tensors.
Three phases (PSUM cannot hold lin, SBUF cannot hold it in fp32):
  PH1 stage-B matmuls + abs-max from PSUM -> AllReduce(max) -> s1
  PH2 stage-B recompute -> q1 (bf16 ints, resident) -> conv (+bias row
      matmul) -> abs-max -> AllReduce(max) -> s3
  PH3 conv recompute -> q2 -> final matmul -> logits out.
"""
import sys
import numpy as np

sys.path.insert(0, "/opt/trn_rl_repo")

import ml_dtypes  # noqa: E402

# Preload the heavy machinery at import time so kernel() itself is lean:
# jax + PJRT/axon client init, bass_rust, and the persistent compile cache.
import os as _os  # noqa: E402
try:
    import antenv.axon_hooks  # noqa: F401,E402
except ImportError:
    # axon NTFF profiling hook unavailable here: a trace request would
    # crash inside run_bass_kernel_spmd, so force tracing off.
    _os.environ["BASS_NEVER_TRACE"] = "1"
try:
    import jax  # noqa: E402
    jax.config.update("jax_compilation_cache_dir", "/tmp/jax_pcache")
    jax.config.update("jax_persistent_cache_min_compile_time_secs", 0.0)
    jax.config.update("jax_persistent_cache_min_entry_size_bytes", -1)
    jax.devices()  # initialize the PJRT client eagerly
except Exception:
    pass
import concourse.bacc  # noqa: F401,E402
from concourse import bass_utils  # noqa: E402

N_CORES = 8
BATCH = 4096
BS = BATCH // N_CORES          # 512 per-core batch shard
MD = 384                        # model dim / channels
KK = 15                         # conv kernel taps
PAD = 7
L = 28                          # sequence length
NG = 12                         # channel groups of 32
NS = 7                          # l-slabs of 4
F32 = np.float32

_M = F32(12582912.0)            # 1.5 * 2^23 : (x+M)-M == round-half-even(x)


def _rne(x):
    return (x.astype(F32) + _M) - _M


def _scale(absmax, bits):
    qmax = F32(2 ** (bits - 1) - 1)
    return np.maximum(F32(absmax) / qmax, F32(1e-8))


def _quant_weight(w, bits):
    s = _scale(np.abs(w).max(), bits)
    q = _rne(w / s).astype(F32)
    return q, s


def _pidx(l_sub, c_sub):
    return l_sub * 32 + c_sub


def bf16(x):
    return np.asarray(x).astype(ml_dtypes.bfloat16).astype(F32)


def _build_host_consts(W1, b1, Wc, bc, Wf, bf, k1):
    """Quantize weights and pack every constant in device layout."""
    bf16 = ml_dtypes.bfloat16
    qW1, _sW1 = _quant_weight(W1, 3)          # [384, 28]
    qWc, sWc = _quant_weight(Wc, 3)           # [384, 1, 15]
    qWf, sWf = _quant_weight(Wf, 3)           # [10, 28*384]

    # stage-B lhsT: [114, 12*128]: row (lp*28+f), col g*128 + pidx(lp, c);
    # rows 112/113 = bf16-hi/lo split of b1/k1 (bias via the qx ones-rows)
    w1 = np.zeros((114, NG * 128), np.float32)
    for g in range(NG):
        for lp in range(4):
            for c in range(32):
                w1[lp * 28:(lp + 1) * 28, g * 128 + _pidx(lp, c)] = \
                    qW1[g * 32 + c, :]
                r = F32(b1[g * 32 + c] / k1)
                hi = F32(bf16(r))
                w1[112, g * 128 + _pidx(lp, c)] = hi
                w1[113, g * 128 + _pidx(lp, c)] = F32(r - hi)
    # conv block-Toeplitz: [128, 5*12*128]:
    #   W_{d,g}[pidx(li,c), pidx(lo,c)] = qWc[c, li - lo + 4d + 7]
    wc = np.zeros((128, 5 * NG * 128), np.float32)
    for d in range(-2, 3):
        for g in range(NG):
            col0 = ((d + 2) * NG + g) * 128
            for li in range(4):
                for lo in range(4):
                    k = li - lo + 4 * d + PAD
                    if 0 <= k < KK:
                        for c in range(32):
                            wc[_pidx(li, c), col0 + _pidx(lo, c)] = \
                                qWc[g * 32 + c, 0, k]
    # conv bias row (used with a device 1/k3 row): [1, 12*128]
    bcr = np.zeros((1, NG * 128), np.float32)
    for g in range(NG):
        for lp in range(4):
            for c in range(32):
                bcr[0, g * 128 + _pidx(lp, c)] = bc[g * 32 + c]
    # final lhsT: [128, 7*12*10]: row pidx(lp,c) of (s,g)-chunk, col j
    wfq = qWf.reshape(10, L, MD)
    wf = np.zeros((128, NS * NG * 10), np.float32)
    for s in range(NS):
        for g in range(NG):
            col0 = (s * NG + g) * 10
            for lp in range(4):
                for c in range(32):
                    wf[_pidx(lp, c), col0:col0 + 10] = \
                        wfq[:, s * 4 + lp, g * 32 + c]
    # per-partition bc columns (PH3 drain bias), one per channel-group g
    bcp = np.zeros((128, NG), np.float32)
    for g in range(NG):
        for lp in range(4):
            for c in range(32):
                bcp[_pidx(lp, c), g] = bc[g * 32 + c]
    bfp = np.zeros((128, 1), np.float32)
    bfp[:10, 0] = bf
    consts = {
        "w1": np.ascontiguousarray(w1.astype(bf16)),
        "wc": np.ascontiguousarray(wc.astype(bf16)),
        "bcr": np.ascontiguousarray(bcr.astype(bf16)),
        "wf": np.ascontiguousarray(wf.astype(bf16)),
        "bcp": bcp, "bfp": bfp,
    }
    return consts, sWc, sWf


_NC_CACHE = {}


def _build_kernel(k1, inv_s0, sWc, sWf):
    """Trace the Bass/Tile kernel. k1 = s0*sW1, inv_s0 host-known floats."""
    key = (k1, inv_s0, sWc, sWf)
    if key in _NC_CACHE:
        return _NC_CACHE[key]
    import concourse.bacc as bacc
    import concourse.tile as tile
    import concourse.mybir as mybir

    dt = mybir.dt
    ALU = mybir.AluOpType
    AFT = mybir.ActivationFunctionType
    AXL = mybir.AxisListType

    nc = bacc.Bacc("TRN2", target_bir_lowering=False, debug=False,
                   num_devices=N_CORES)

    # consolidated inputs: 2 tensors/core (axon device_put latency dominates)
    CB = NG * 128              # 1536
    CW = 5 * NG * 128          # 7680
    CF = NS * NG * 10          # 840
    cb_d = nc.dram_tensor("cb", [128, CB + CW + CB + CF], dt.bfloat16,
                          kind="ExternalInput")
    cf_d = nc.dram_tensor("cf", [128, NS * BS + NG + 1], dt.float32,
                          kind="ExternalInput")
    w1_d = cb_d[0:114, 0:CB]
    wc_d = cb_d[:, CB:CB + CW]
    bcr_d = cb_d[0:1, CB + CW:CB + CW + CB]
    wf_d = cb_d[:, CB + CW + CB:CB + CW + CB + CF]
    x_d = cf_d[0:114, 0:NS * BS]
    bc_d = cf_d[:, NS * BS:NS * BS + NG]
    bf_d = cf_d[:, NS * BS + NG:NS * BS + NG + 1]
    out_d = nc.dram_tensor("out", [10, BS], dt.float32, kind="ExternalOutput")

    rg = [list(range(N_CORES))]

    with tile.TileContext(nc) as tc:
        with (
            tc.tile_pool(name="const", bufs=1) as cpool,
            tc.tile_pool(name="xio", bufs=1) as xpool,
            tc.tile_pool(name="work", bufs=2) as wpool,
            tc.tile_pool(name="scal", bufs=1) as spool,
            tc.tile_pool(name="ps1", bufs=2, space="PSUM") as ps1,
            tc.tile_pool(name="ps3", bufs=2, space="PSUM") as ps3,
            tc.tile_pool(name="psf", bufs=1, space="PSUM") as psf,
            tc.tile_pool(name="psb", bufs=1, space="PSUM") as psb,
            tc.tile_pool(name="dram", bufs=1, space="DRAM") as dpool,
        ):
            # ---- constants into SBUF
            w1_t = cpool.tile([114, NG * 128], dt.bfloat16)
            wc_t = cpool.tile([128, 5 * NG * 128], dt.bfloat16)
            bcr_t = cpool.tile([1, NG * 128], dt.bfloat16)
            wf_t = cpool.tile([128, NS * NG * 10], dt.bfloat16)
            bc_t = cpool.tile([128, NG], dt.float32)
            bf_t = cpool.tile([128, 1], dt.float32)
            ones_r = cpool.tile([1, 128], dt.float32)     # bcast lhsT
            ones_b = cpool.tile([1, BS], dt.float32)      # bias-mm rhs helper
            nc.sync.dma_start(w1_t, w1_d)
            nc.sync.dma_start(wc_t, wc_d)
            nc.sync.dma_start(bcr_t, bcr_d)
            nc.sync.dma_start(wf_t, wf_d)
            nc.sync.dma_start(bc_t, bc_d)
            nc.sync.dma_start(bf_t, bf_d)
            nc.gpsimd.memset(ones_r, 1.0)
            nc.gpsimd.memset(ones_b, 1.0)
            mM_t = cpool.tile([128, 1], dt.float32)
            nc.gpsimd.memset(mM_t, float(_M))

            # ---- stage A: load + quantize input (qx = rne(x/s0), bf16 ints)
            xt = xpool.tile([114, NS * BS], dt.float32)
            nc.sync.dma_start(xt, x_d)
            x2 = xpool.tile([114, NS * BS], dt.float32)
            nc.vector.tensor_scalar(x2, xt, float(inv_s0), float(_M),
                                    ALU.mult, ALU.add)
            qx = cpool.tile([114, NS * BS], dt.bfloat16)
            nc.vector.tensor_scalar(qx, x2, float(-_M), None, ALU.add)

            q1_t = cpool.tile([128, NG * NS * BS], dt.bfloat16)
            mbuf = spool.tile([128, NG * NS], dt.float32)
            m3buf = spool.tile([128, NG * NS], dt.float32)

            def stage_b_mm(g, s):
                p = ps1.tile([128, BS], dt.float32, tag="ps1", name=f"p1_{g}_{s}")
                nc.tensor.matmul(p, w1_t[0:114, g * 128:(g + 1) * 128],
                                 qx[0:114, s * BS:(s + 1) * BS],
                                 start=True, stop=True)
                return p

            def conv_mm(g, s, bias_rhs=None):
                p3 = ps3.tile([128, BS], dt.float32, tag="ps3",
                              name=f"p3_{g}_{s}")
                dmin = max(-2, -s)
                dmax = min(2, (NS - 1) - s)
                for d in range(dmin, dmax + 1):
                    col0 = ((d + 2) * NG + g) * 128
                    nc.tensor.matmul(
                        p3, wc_t[:, col0:col0 + 128],
                        q1_t[:, (g * NS + s + d) * BS:(g * NS + s + d + 1) * BS],
                        start=(d == dmin), stop=(d == dmax and bias_rhs is None))
                if bias_rhs is not None:
                    nc.tensor.matmul(p3, bcr_t[0:1, g * 128:(g + 1) * 128],
                                     bias_rhs, start=False, stop=True)
                return p3

            # ---------------- PH1: abs-max of stage-B psum -----------------
            for g in range(NG):
                for s in range(NS):
                    p = stage_b_mm(g, s)
                    nc.vector.tensor_reduce(
                        mbuf[:, g * NS + s: g * NS + s + 1], p, axis=AXL.X,
                        op=ALU.max, apply_absolute_value=True)

            mred = spool.tile([128, 1], dt.float32)
            nc.vector.tensor_reduce(mred, mbuf, axis=AXL.X, op=ALU.max)
            m1s = spool.tile([1, 8], dt.float32)
            nc.gpsimd.memset(m1s, 0.0)
            nc.gpsimd.tensor_reduce(m1s[0:1, 0:1], mred, axis=AXL.C, op=ALU.max)

            ar_in1 = dpool.tile([1, 8], dt.float32)
            ar_out1 = dpool.tile([1, 8], dt.float32, addr_space="Shared")
            nc.sync.dma_start(ar_in1, m1s)
            nc.gpsimd.collective_compute(
                "AllReduce", ALU.max, ins=[ar_in1.opt()], outs=[ar_out1.opt()],
                replica_groups=rg)
            m1g = spool.tile([1, 8], dt.float32)
            nc.sync.dma_start(m1g, ar_out1[:])

            # broadcast global max to [128,1] via ones-lhsT matmul
            pb = psb.tile([128, 1], dt.float32, tag="pb", name="pb1")
            nc.tensor.matmul(pb, ones_r, m1g[0:1, 0:1], start=True, stop=True)
            m1t = spool.tile([128, 1], dt.float32)
            nc.scalar.activation(m1t, pb, AFT.Copy)

            # scalar chain 1 (m1t = max|raw+b1/k1| -> s1 = max(m*k1/127,1e-8))
            s1_t = spool.tile([128, 1], dt.float32)
            nc.vector.tensor_scalar(s1_t, m1t, float(k1 / 127.0), float(1e-8),
                                    ALU.mult, ALU.max)
            inv_s1 = spool.tile([128, 1], dt.float32)
            nc.vector.reciprocal(inv_s1, s1_t)
            a1_t = spool.tile([128, 1], dt.float32)
            nc.vector.tensor_scalar(a1_t, inv_s1, float(k1), None, ALU.mult)
            th1 = spool.tile([128, 1], dt.float32)
            nc.scalar.activation(th1, s1_t, AFT.Tanh, scale=127.0)
            s2_t = spool.tile([128, 1], dt.float32)
            nc.vector.tensor_scalar(s2_t, th1, float(1.0 / 127.0), float(1e-8),
                                    ALU.mult, ALU.max)
            inv_s2 = spool.tile([128, 1], dt.float32)
            nc.vector.reciprocal(inv_s2, s2_t)
            k3_t = spool.tile([128, 1], dt.float32)
            nc.vector.tensor_scalar(k3_t, s2_t, float(sWc), None, ALU.mult)
            inv_k3 = spool.tile([128, 1], dt.float32)
            nc.vector.reciprocal(inv_k3, k3_t)
            # device row [1, BS] of 1/k3 for the conv bias matmul (bf16)
            rk3_f = spool.tile([1, BS], dt.float32)
            nc.vector.scalar_tensor_tensor(rk3_f, ones_b, inv_k3[0:1, 0:1],
                                           ones_b, ALU.mult, ALU.mult)
            rk3 = spool.tile([1, BS], dt.bfloat16)
            nc.vector.tensor_copy(rk3, rk3_f)

            def quant_chain(p, a_ap, bias_ap, sc_ap, invn_ap, qdst, nm):
                """qdst (bf16 ints) = rne(tanh(sc*rne(p*a + bias)) * invn).

                bias_ap may be None when the bias is already inside p (then
                the +M is fused into the ScalarE affine drain)."""
                w = wpool.tile([128, BS], dt.float32, tag="ew", name=f"w{nm}")
                if bias_ap is None:
                    nc.scalar.activation(w, p, AFT.Identity, bias=mM_t,
                                         scale=a_ap)
                    ql = wpool.tile([128, BS], dt.bfloat16, tag="eql",
                                    name=f"ql{nm}")
                    nc.gpsimd.tensor_scalar(ql, w, float(-_M), None, ALU.add)
                else:
                    nc.scalar.activation(w, p, AFT.Identity, bias=bias_ap,
                                         scale=a_ap)
                    ql = wpool.tile([128, BS], dt.bfloat16, tag="eql",
                                    name=f"ql{nm}")
                    nc.vector.tensor_scalar(ql, w, float(_M), float(-_M),
                                            ALU.add, ALU.add)
                t = wpool.tile([128, BS], dt.float32, tag="et", name=f"t{nm}")
                nc.scalar.activation(t, ql, AFT.Tanh, scale=sc_ap)
                v = wpool.tile([128, BS], dt.float32, tag="ev", name=f"v{nm}")
                nc.vector.tensor_scalar(v, t, invn_ap, float(_M),
                                        ALU.mult, ALU.add)
                nc.gpsimd.tensor_scalar(qdst, v, float(-_M), None, ALU.add)

            # ---------------- PH2: q1, conv(+bias), abs-max ----------------
            for g in range(NG):
                for s in range(NS):
                    p = stage_b_mm(g, s)
                    quant_chain(p, a1_t, None, s1_t, inv_s2,
                                q1_t[:, (g * NS + s) * BS:(g * NS + s + 1) * BS],
                                f"b{g}_{s}")
            for g in range(NG):
                for s in range(NS):
                    p3 = conv_mm(g, s, bias_rhs=rk3)
                    nc.vector.tensor_reduce(
                        m3buf[:, g * NS + s: g * NS + s + 1], p3, axis=AXL.X,
                        op=ALU.max, apply_absolute_value=True)

            m3red = spool.tile([128, 1], dt.float32)
            nc.vector.tensor_reduce(m3red, m3buf, axis=AXL.X, op=ALU.max)
            m3s = spool.tile([1, 8], dt.float32)
            nc.gpsimd.memset(m3s, 0.0)
            nc.gpsimd.tensor_reduce(m3s[0:1, 0:1], m3red, axis=AXL.C, op=ALU.max)

            ar_in2 = dpool.tile([1, 8], dt.float32)
            ar_out2 = dpool.tile([1, 8], dt.float32, addr_space="Shared")
            nc.sync.dma_start(ar_in2, m3s)
            nc.gpsimd.collective_compute(
                "AllReduce", ALU.max, ins=[ar_in2.opt()], outs=[ar_out2.opt()],
                replica_groups=rg)
            m3g = spool.tile([1, 8], dt.float32)
            nc.sync.dma_start(m3g, ar_out2[:])
            pb3 = psb.tile([128, 1], dt.float32, tag="pb", name="pb3")
            nc.tensor.matmul(pb3, ones_r, m3g[0:1, 0:1], start=True, stop=True)
            m3t = spool.tile([128, 1], dt.float32)
            nc.scalar.activation(m3t, pb3, AFT.Copy)

            # scalar chain 2: m3 = max|raw3+bc/k3| -> s3 = max(m3*k3/127,1e-8)
            s3_t = spool.tile([128, 1], dt.float32)
            nc.vector.tensor_mul(s3_t, m3t, k3_t)
            nc.vector.tensor_scalar(s3_t, s3_t, float(1.0 / 127.0), float(1e-8),
                                    ALU.mult, ALU.max)
            inv_s3 = spool.tile([128, 1], dt.float32)
            nc.vector.reciprocal(inv_s3, s3_t)
            a3_t = spool.tile([128, 1], dt.float32)
            nc.vector.tensor_mul(a3_t, k3_t, inv_s3)
            th3 = spool.tile([128, 1], dt.float32)
            nc.scalar.activation(th3, s3_t, AFT.Tanh, scale=127.0)
            s4_t = spool.tile([128, 1], dt.float32)
            nc.vector.tensor_scalar(s4_t, th3, float(1.0 / 127.0), float(1e-8),
                                    ALU.mult, ALU.max)
            inv_s4 = spool.tile([128, 1], dt.float32)
            nc.vector.reciprocal(inv_s4, s4_t)
            k5_t = spool.tile([128, 1], dt.float32)
            nc.vector.tensor_scalar(k5_t, s4_t, float(sWf), None, ALU.mult)
            bcs3 = spool.tile([128, NG], dt.float32)
            for g in range(NG):
                nc.vector.tensor_mul(bcs3[:, g:g + 1], bc_t[:, g:g + 1], inv_s3)

            # ---------------- PH3: conv recompute, q2, final ---------------
            pf = psf.tile([10, BS], dt.float32)
            n_acc = NG * NS
            idx = 0
            for g in range(NG):
                for s in range(NS):
                    p3 = conv_mm(g, s)
                    q2 = wpool.tile([128, BS], dt.bfloat16, tag="q2",
                                    name=f"q2_{g}_{s}")
                    quant_chain(p3, a3_t, bcs3[:, g:g + 1], s3_t, inv_s4, q2,
                                f"d{g}_{s}")
                    col0 = (s * NG + g) * 10
                    nc.tensor.matmul(pf, wf_t[:, col0:col0 + 10], q2,
                                     start=(idx == 0), stop=(idx == n_acc - 1),
                                     skip_group_check=True)
                    idx += 1

            lg_sb = wpool.tile([10, BS], dt.float32, tag="lg")
            nc.vector.tensor_scalar(lg_sb, pf, k5_t[0:10, 0:1],
                                    bf_t[0:10, 0:1], ALU.mult, ALU.add)
            nc.sync.dma_start(out_d[:], lg_sb)

    nc.compile()
    _NC_CACHE[(k1, inv_s0, sWc, sWf)] = nc
    return nc


def kernel(image, W1, b1, Wc, bc, Wf, bf):

    image = np.asarray(image, F32)
    W1 = np.asarray(W1, F32)

    _qW1, sW1 = _quant_weight(W1, 3)
    s0 = _scale(np.abs(image).max(), 8)      # host: exact global input scale
    k1 = float(s0 * sW1)

    consts, sWc, sWf = _build_host_consts(
        W1, np.asarray(b1, F32), np.asarray(Wc, F32),
        np.asarray(bc, F32), np.asarray(Wf, F32), np.asarray(bf, F32), k1)

    nc = _build_kernel(k1, float(1.0 / s0), float(sWc), float(sWf))

    # host input layout: [b, l, f] -> per core [lp, f, lg, b] = [112, 7*512]
    img = image.reshape(N_CORES, BS, NS, 4, L)      # [core, b, lg, lp, f]
    img = img.transpose(0, 3, 4, 2, 1)               # [core, lp, f, lg, b]
    img = img.reshape(N_CORES, 112, NS * BS)
    ones_row = np.full((N_CORES, 2, NS * BS), s0, F32)  # quantizes to 1
    img = np.ascontiguousarray(np.concatenate([img, ones_row], axis=1), F32)

    CB, CW, CF = NG * 128, 5 * NG * 128, NS * NG * 10
    cb = np.zeros((128, CB + CW + CB + CF), ml_dtypes.bfloat16)
    cb[0:114, 0:CB] = consts["w1"]
    cb[:, CB:CB + CW] = consts["wc"]
    cb[0:1, CB + CW:CB + CW + CB] = consts["bcr"]
    cb[:, CB + CW + CB:] = consts["wf"]
    in_maps = []
    for i in range(N_CORES):
        cf = np.zeros((128, NS * BS + NG + 1), F32)
        cf[0:114, 0:NS * BS] = img[i]
        cf[:, NS * BS:NS * BS + NG] = consts["bcp"]
        cf[:, NS * BS + NG:] = consts["bfp"]
        in_maps.append({"cb": cb, "cf": np.ascontiguousarray(cf)})

    res = bass_utils.run_bass_kernel_spmd(nc, in_maps,
                                          core_ids=list(range(N_CORES)))
    kernel.last_results = res

    # gather + host-side exact final fake-quant (s5 global)
    logits = np.stack([r["out"] for r in res.results])   # [8, 10, 512]
    logits = logits.transpose(0, 2, 1).reshape(BATCH, 10)
    s5 = _scale(np.abs(logits).max(), 8)
    return (_rne(logits / s5) * s5).astype(F32)



# revision 5
# speedup vs baseline: 6.9976x; 6.9976x over previous
"""nn_ConvModel — Bass/Tile kernel, data-parallel over 8 TRN2 NeuronCores.

Strategy (per sharding_hint): batch dim of `image` sharded 8 ways, tiny
3-bit-quantized weights replicated on device via an in-kernel AllGather
(each core ships only a 1/8 row-chunk of the packed weight matrix, so the
slow host->device axon link carries each weight byte once).  The two
data-dependent activation quant scales (s1 for lin, s3 for the depthwise
conv output) are computed on-device as shard-local abs-maxes +
AllReduce(max).  The input scale s0 (pure function of the input) is
applied on the host: the image ships pre-quantized as bf16 integers.  The
final logits scale s5 (160 KB tensor) is applied on the host, exactly.

All input-dependent scalars (k1 = s0*sW1, sWc, sWf, biases) enter the
device as tensors (bf16 hi/lo pairs reassembled to fp32 on device), so
the Bass program itself is input-value-independent: it is traced,
compiled and warm-executed once at import time, and kernel() only packs
inputs, runs the persistent jitted executable, and unpacks.

Device layout (per core, batch shard b=512):
  partitions = (l%4)*32 + channel%32   [l = sequence pos 0..27, 28=7*4]
  free       = batch
  * stage-B linear:  lhsT[(f28,lp4)+ones=113, (l4,c32)=128] block-diag in
    l with the bias row b1/k1 folded in; one matmul per (channel-group g
    of 12, l-slab s of 7), N=512.
  * depthwise conv:  block-Toeplitz 128x128 weights W_d (d=-2..2), ~29
    accumulated TensorE matmuls per (g,s); no transposes anywhere.
  * final linear:    Wf rearranged to the same (l4,c32) partition order,
    84 accumulating matmuls into one [10,512] PSUM tile.
  * fake-quant rounding = (+1.5*2^23, -1.5*2^23) round-to-nearest-even,
    spread across ScalarE, VectorE and GpSimd; tanh on ScalarE.
Three phases (PSUM cannot hold lin, SBUF cannot hold it in fp32):
  PH1 stage-B matmuls + abs-max from PSUM -> AllReduce(max) -> s1
  PH2 stage-B recompute -> q1 (bf16 ints, resident) -> conv (+bias row
      matmul) -> abs-max -> AllReduce(max) -> s3
  PH3 conv recompute -> q2 -> final matmul -> logits out.
"""
import sys
import os as _os
import numpy as np

sys.path.insert(0, "/opt/trn_rl_repo")

import ml_dtypes  # noqa: E402

try:
    import antenv.axon_hooks  # noqa: F401,E402
except ImportError:
    # axon NTFF profiling hook unavailable here: a trace request would
    # crash inside the axon run path, so force tracing off.
    _os.environ["BASS_NEVER_TRACE"] = "1"
import jax  # noqa: E402
try:
    jax.config.update("jax_compilation_cache_dir", "/tmp/jax_pcache")
    jax.config.update("jax_persistent_cache_min_compile_time_secs", 0.0)
    jax.config.update("jax_persistent_cache_min_entry_size_bytes", -1)
except Exception:
    pass
jax.devices()  # initialize the PJRT client eagerly

import concourse.bacc as bacc  # noqa: E402
import concourse.tile as tile  # noqa: E402
import concourse.mybir as mybir  # noqa: E402
from concourse import bass2jax  # noqa: E402
from jax.experimental.shard_map import shard_map  # noqa: E402
from jax.sharding import Mesh, PartitionSpec, NamedSharding  # noqa: E402

N_CORES = 8
BATCH = 4096
BS = BATCH // N_CORES          # 512 per-core batch shard
MD = 384                        # model dim / channels
KK = 15                         # conv kernel taps
PAD = 7
L = 28                          # sequence length
NG = 12                         # channel groups of 32
NS = 7                          # l-slabs of 4
F32 = np.float32
BF16 = ml_dtypes.bfloat16

_M = F32(12582912.0)            # 1.5 * 2^23 : (x+M)-M == round-half-even(x)

# single bf16 input tensor [128, IC] per core:
QC = NS * BS                    # 3584  qx cols (rows 0..113 used)
WCH0 = QC                       # weight-chunk cols (1/8 of wg, reshaped)
WW = 5 * NG * 128 + NS * NG * 10 + NG * 128   # 7680 + 840 + 1536 = 10056
WCHW = WW * 16 // 128           # 1257
SC0 = WCH0 + WCHW               # 4841  scalar hi/lo cols
NSC = 17                        # k1, k1/127, sWc, sWf, bcp[12], bfp
IC = SC0 + 2 * NSC              # 4875
# wg (gathered weights) column layout
WCOFF = 0                       # conv block-Toeplitz  [128, 7680]
WFOFF = 5 * NG * 128            # final linear         [128, 840]
W1OFF = WFOFF + NS * NG * 10    # stage-B block + bias rows + bcr [115, 1536]


def _rne(x):
    return (x.astype(F32) + _M) - _M


def _scale(absmax, bits):
    qmax = F32(2 ** (bits - 1) - 1)
    return np.maximum(F32(absmax) / qmax, F32(1e-8))


def _quant_weight(w, bits):
    s = _scale(np.abs(w).max(), bits)
    q = _rne(w / s).astype(F32)
    return q, s


def _build_nc():
    """Trace the input-value-independent Bass/Tile kernel."""
    dt = mybir.dt
    ALU = mybir.AluOpType
    AFT = mybir.ActivationFunctionType
    AXL = mybir.AxisListType

    nc = bacc.Bacc("TRN2", target_bir_lowering=False, debug=False,
                   num_devices=N_CORES)

    inp_d = nc.dram_tensor("inp", [128, IC], dt.bfloat16,
                           kind="ExternalInput")
    out_d = nc.dram_tensor("out", [10, BS], dt.float32, kind="ExternalOutput")

    rg = [list(range(N_CORES))]

    with tile.TileContext(nc) as tc:
        with (
            tc.tile_pool(name="const", bufs=1) as cpool,
            tc.tile_pool(name="work", bufs=2) as wpool,
            tc.tile_pool(name="scal", bufs=1) as spool,
            tc.tile_pool(name="ps1", bufs=2, space="PSUM") as ps1,
            tc.tile_pool(name="ps3", bufs=2, space="PSUM") as ps3,
            tc.tile_pool(name="psf", bufs=1, space="PSUM") as psf,
            tc.tile_pool(name="psb", bufs=1, space="PSUM") as psb,
            tc.tile_pool(name="dram", bufs=1, space="DRAM") as dpool,
        ):
            # ---- weight AllGather: each core contributed rows 16r..16r+16
            # of the packed [128, WW] weight matrix as a [128, WCHW] blob.
            # Collective inputs must be contiguous in DRAM: stage the
            # strided column block through a contiguous DRAM tile first.
            wch_t = dpool.tile([128, WCHW], dt.bfloat16)
            nc.sync.dma_start(wch_t, inp_d[:, WCH0:WCH0 + WCHW])
            wg_t = dpool.tile([128, WW], dt.bfloat16, addr_space="Shared")
            nc.gpsimd.collective_compute(
                "AllGather", ALU.bypass,
                ins=[wch_t.opt()], outs=[wg_t.opt()],
                replica_groups=rg)

            # ---- per-core inputs straight from DRAM
            qx = cpool.tile([114, QC], dt.bfloat16)
            nc.sync.dma_start(qx, inp_d[0:114, 0:QC])
            hl_t = cpool.tile([128, 2 * NSC], dt.bfloat16)
            nc.sync.dma_start(hl_t, inp_d[:, SC0:SC0 + 2 * NSC])
            sc_t = cpool.tile([128, NSC], dt.float32)
            nc.vector.tensor_add(sc_t, hl_t[:, 0:NSC], hl_t[:, NSC:2 * NSC])
            k1_c = sc_t[:, 0:1]
            k1d127_c = sc_t[:, 1:2]
            sWc_c = sc_t[:, 2:3]
            sWf_c = sc_t[:, 3:4]
            bc_t = sc_t[:, 4:16]
            bf_c = sc_t[:, 16:17]

            # ---- gathered weights into SBUF
            wc_t = cpool.tile([128, 5 * NG * 128], dt.bfloat16)
            nc.sync.dma_start(wc_t, wg_t[:, WCOFF:WCOFF + 5 * NG * 128])
            wf_t = cpool.tile([128, NS * NG * 10], dt.bfloat16)
            nc.sync.dma_start(wf_t, wg_t[:, WFOFF:WFOFF + NS * NG * 10])
            w1_t = cpool.tile([114, NG * 128], dt.bfloat16)
            nc.sync.dma_start(w1_t, wg_t[0:114, W1OFF:W1OFF + NG * 128])
            bcr_t = cpool.tile([1, NG * 128], dt.bfloat16)
            nc.sync.dma_start(bcr_t, wg_t[114:115, W1OFF:W1OFF + NG * 128])

            ones_r = cpool.tile([1, 128], dt.float32)     # bcast lhsT
            ones_b = cpool.tile([1, BS], dt.float32)      # bias-mm rhs helper
            nc.gpsimd.memset(ones_r, 1.0)
            nc.gpsimd.memset(ones_b, 1.0)
            mM_t = cpool.tile([128, 1], dt.float32)
            nc.gpsimd.memset(mM_t, float(_M))

            q1_t = cpool.tile([128, NG * NS * BS], dt.bfloat16)
            mbuf = spool.tile([128, NG * NS], dt.float32)
            m3buf = spool.tile([128, NG * NS], dt.float32)

            def stage_b_mm(g, s):
                p = ps1.tile([128, BS], dt.float32, tag="ps1", name=f"p1_{g}_{s}")
                nc.tensor.matmul(p, w1_t[0:114, g * 128:(g + 1) * 128],
                                 qx[0:114, s * BS:(s + 1) * BS],
                                 start=True, stop=True)
                return p

            def conv_mm(g, s, bias_rhs=None):
                p3 = ps3.tile([128, BS], dt.float32, tag="ps3",
                              name=f"p3_{g}_{s}")
                dmin = max(-2, -s)
                dmax = min(2, (NS - 1) - s)
                for d in range(dmin, dmax + 1):
                    col0 = ((d + 2) * NG + g) * 128
                    nc.tensor.matmul(
                        p3, wc_t[:, col0:col0 + 128],
                        q1_t[:, (g * NS + s + d) * BS:(g * NS + s + d + 1) * BS],
                        start=(d == dmin), stop=(d == dmax and bias_rhs is None))
                if bias_rhs is not None:
                    nc.tensor.matmul(p3, bcr_t[0:1, g * 128:(g + 1) * 128],
                                     bias_rhs, start=False, stop=True)
                return p3

            # ---------------- PH1: abs-max of stage-B psum -----------------
            for g in range(NG):
                for s in range(NS):
                    p = stage_b_mm(g, s)
                    nc.vector.tensor_reduce(
                        mbuf[:, g * NS + s: g * NS + s + 1], p, axis=AXL.X,
                        op=ALU.max, apply_absolute_value=True)

            mred = spool.tile([128, 1], dt.float32)
            nc.vector.tensor_reduce(mred, mbuf, axis=AXL.X, op=ALU.max)
            m1s = spool.tile([1, 8], dt.float32)
            nc.gpsimd.memset(m1s, 0.0)
            nc.gpsimd.tensor_reduce(m1s[0:1, 0:1], mred, axis=AXL.C, op=ALU.max)

            ar_in1 = dpool.tile([1, 8], dt.float32)
            ar_out1 = dpool.tile([1, 8], dt.float32, addr_space="Shared")
            nc.sync.dma_start(ar_in1, m1s)
            nc.gpsimd.collective_compute(
                "AllReduce", ALU.max, ins=[ar_in1.opt()], outs=[ar_out1.opt()],
                replica_groups=rg)
            m1g = spool.tile([1, 8], dt.float32)
            nc.sync.dma_start(m1g, ar_out1[:])

            # broadcast global max to [128,1] via ones-lhsT matmul
            pb = psb.tile([128, 1], dt.float32, tag="pb", name="pb1")
            nc.tensor.matmul(pb, ones_r, m1g[0:1, 0:1], start=True, stop=True)
            m1t = spool.tile([128, 1], dt.float32)
            nc.scalar.activation(m1t, pb, AFT.Copy)

            # scalar chain 1 (m1t = max|raw+b1/k1| -> s1 = max(m*k1/127,1e-8))
            s1_t = spool.tile([128, 1], dt.float32)
            nc.vector.tensor_mul(s1_t, m1t, k1d127_c)
            nc.vector.tensor_scalar(s1_t, s1_t, float(1e-8), None, ALU.max)
            inv_s1 = spool.tile([128, 1], dt.float32)
            nc.vector.reciprocal(inv_s1, s1_t)
            a1_t = spool.tile([128, 1], dt.float32)
            nc.vector.tensor_mul(a1_t, inv_s1, k1_c)
            th1 = spool.tile([128, 1], dt.float32)
            nc.scalar.activation(th1, s1_t, AFT.Tanh, scale=127.0)
            s2_t = spool.tile([128, 1], dt.float32)
            nc.vector.tensor_scalar(s2_t, th1, float(1.0 / 127.0), float(1e-8),
                                    ALU.mult, ALU.max)
            inv_s2 = spool.tile([128, 1], dt.float32)
            nc.vector.reciprocal(inv_s2, s2_t)
            k3_t = spool.tile([128, 1], dt.float32)
            nc.vector.tensor_mul(k3_t, s2_t, sWc_c)
            inv_k3 = spool.tile([128, 1], dt.float32)
            nc.vector.reciprocal(inv_k3, k3_t)
            # device row [1, BS] of 1/k3 for the conv bias matmul (bf16)
            rk3_f = spool.tile([1, BS], dt.float32)
            nc.vector.scalar_tensor_tensor(rk3_f, ones_b, inv_k3[0:1, 0:1],
                                           ones_b, ALU.mult, ALU.mult)
            rk3 = spool.tile([1, BS], dt.bfloat16)
            nc.vector.tensor_copy(rk3, rk3_f)

            def quant_chain(p, a_ap, bias_ap, sc_ap, invn_ap, qdst, nm):
                """qdst (bf16 ints) = rne(tanh(sc*rne(p*a + bias)) * invn).

                bias_ap may be None when the bias is already inside p (then
                the +M is fused into the ScalarE affine drain)."""
                w = wpool.tile([128, BS], dt.float32, tag="ew", name=f"w{nm}")
                if bias_ap is None:
                    nc.scalar.activation(w, p, AFT.Identity, bias=mM_t,
                                         scale=a_ap)
                    ql = wpool.tile([128, BS], dt.bfloat16, tag="eql",
                                    name=f"ql{nm}")
                    nc.gpsimd.tensor_scalar(ql, w, float(-_M), None, ALU.add)
                else:
                    nc.scalar.activation(w, p, AFT.Identity, bias=bias_ap,
                                         scale=a_ap)
                    ql = wpool.tile([128, BS], dt.bfloat16, tag="eql",
                                    name=f"ql{nm}")
                    nc.vector.tensor_scalar(ql, w, float(_M), float(-_M),
                                            ALU.add, ALU.add)
                t = wpool.tile([128, BS], dt.float32, tag="et", name=f"t{nm}")
                nc.scalar.activation(t, ql, AFT.Tanh, scale=sc_ap)
                v = wpool.tile([128, BS], dt.float32, tag="ev", name=f"v{nm}")
                nc.vector.tensor_scalar(v, t, invn_ap, float(_M),
                                        ALU.mult, ALU.add)
                nc.gpsimd.tensor_scalar(qdst, v, float(-_M), None, ALU.add)

            # ---------------- PH2: q1, conv(+bias), abs-max ----------------
            for g in range(NG):
                for s in range(NS):
                    p = stage_b_mm(g, s)
                    quant_chain(p, a1_t, None, s1_t, inv_s2,
                                q1_t[:, (g * NS + s) * BS:(g * NS + s + 1) * BS],
                                f"b{g}_{s}")
            for g in range(NG):
                for s in range(NS):
                    p3 = conv_mm(g, s, bias_rhs=rk3)
                    nc.vector.tensor_reduce(
                        m3buf[:, g * NS + s: g * NS + s + 1], p3, axis=AXL.X,
                        op=ALU.max, apply_absolute_value=True)

            m3red = spool.tile([128, 1], dt.float32)
            nc.vector.tensor_reduce(m3red, m3buf, axis=AXL.X, op=ALU.max)
            m3s = spool.tile([1, 8], dt.float32)
            nc.gpsimd.memset(m3s, 0.0)
            nc.gpsimd.tensor_reduce(m3s[0:1, 0:1], m3red, axis=AXL.C, op=ALU.max)

            ar_in2 = dpool.tile([1, 8], dt.float32)
            ar_out2 = dpool.tile([1, 8], dt.float32, addr_space="Shared")
            nc.sync.dma_start(ar_in2, m3s)
            nc.gpsimd.collective_compute(
                "AllReduce", ALU.max, ins=[ar_in2.opt()], outs=[ar_out2.opt()],
                replica_groups=rg)
            m3g = spool.tile([1, 8], dt.float32)
            nc.sync.dma_start(m3g, ar_out2[:])
            pb3 = psb.tile([128, 1], dt.float32, tag="pb", name="pb3")
            nc.tensor.matmul(pb3, ones_r, m3g[0:1, 0:1], start=True, stop=True)
            m3t = spool.tile([128, 1], dt.float32)
            nc.scalar.activation(m3t, pb3, AFT.Copy)

            # scalar chain 2: m3 = max|raw3+bc/k3| -> s3 = max(m3*k3/127,1e-8)
            s3_t = spool.tile([128, 1], dt.float32)
            nc.vector.tensor_mul(s3_t, m3t, k3_t)
            nc.vector.tensor_scalar(s3_t, s3_t, float(1.0 / 127.0), float(1e-8),
                                    ALU.mult, ALU.max)
            inv_s3 = spool.tile([128, 1], dt.float32)
            nc.vector.reciprocal(inv_s3, s3_t)
            a3_t = spool.tile([128, 1], dt.float32)
            nc.vector.tensor_mul(a3_t, k3_t, inv_s3)
            th3 = spool.tile([128, 1], dt.float32)
            nc.scalar.activation(th3, s3_t, AFT.Tanh, scale=127.0)
            s4_t = spool.tile([128, 1], dt.float32)
            nc.vector.tensor_scalar(s4_t, th3, float(1.0 / 127.0), float(1e-8),
                                    ALU.mult, ALU.max)
            inv_s4 = spool.tile([128, 1], dt.float32)
            nc.vector.reciprocal(inv_s4, s4_t)
            k5_t = spool.tile([128, 1], dt.float32)
            nc.vector.tensor_mul(k5_t, s4_t, sWf_c)
            bcs3 = spool.tile([128, NG], dt.float32)
            for g in range(NG):
                nc.vector.tensor_mul(bcs3[:, g:g + 1], bc_t[:, g:g + 1], inv_s3)

            # ---------------- PH3: conv recompute, q2, final ---------------
            pf = psf.tile([10, BS], dt.float32)
            n_acc = NG * NS
            idx = 0
            for g in range(NG):
                for s in range(NS):
                    p3 = conv_mm(g, s)
                    q2 = wpool.tile([128, BS], dt.bfloat16, tag="q2",
                                    name=f"q2_{g}_{s}")
                    quant_chain(p3, a3_t, bcs3[:, g:g + 1], s3_t, inv_s4, q2,
                                f"d{g}_{s}")
                    col0 = (s * NG + g) * 10
                    nc.tensor.matmul(pf, wf_t[:, col0:col0 + 10], q2,
                                     start=(idx == 0), stop=(idx == n_acc - 1),
                                     skip_group_check=True)
                    idx += 1

            lg_sb = wpool.tile([10, BS], dt.float32, tag="lg")
            nc.vector.tensor_scalar(lg_sb, pf, k5_t[0:10, 0:1],
                                    bf_c[0:10, 0:1], ALU.mult, ALU.add)
            nc.sync.dma_start(out_d[:], lg_sb)

    nc.compile()
    return nc


def _make_runner(nc):
    """Persistent jitted SPMD executable over 8 cores.

    Mirrors concourse.bass2jax.run_bass_via_pjrt's multi-core path, but
    the jit (and hence the traced/lowered/compiled executable) is built
    once at import and reused on every kernel() call.
    """
    bass2jax.install_neuronx_cc_hook()
    partition_name = (nc.partition_id_tensor.name
                      if nc.partition_id_tensor else None)

    in_names, out_names, out_avals, zero_outs = [], [], [], []
    for alloc in nc.m.functions[0].allocations:
        if not isinstance(alloc, mybir.MemoryLocationSet):
            continue
        name = alloc.memorylocations[0].name
        if alloc.kind == "ExternalInput":
            if name != partition_name:
                in_names.append(name)
        elif alloc.kind == "ExternalOutput":
            shape = tuple(alloc.tensor_shape)
            dtype = mybir.dt.np(alloc.dtype)
            out_names.append(name)
            out_avals.append(jax.core.ShapedArray(shape, dtype))
            zero_outs.append(np.zeros(shape, dtype))
    n_params = len(in_names)
    n_outs = len(out_avals)
    in_names = in_names + out_names
    if partition_name is not None:
        in_names.append(partition_name)
    donate = tuple(range(n_params, n_params + n_outs))

    def _body(*args):
        operands = list(args)
        if partition_name is not None:
            operands.append(bass2jax.partition_id_tensor())
        outs = bass2jax._bass_exec_p.bind(
            *operands,
            out_avals=tuple(out_avals),
            in_names=tuple(in_names),
            out_names=tuple(out_names),
            lowering_input_output_aliases=(),
            sim_require_finite=True,
            sim_require_nnan=True,
            nc=nc,
        )
        return tuple(outs)

    devices = jax.devices()[:N_CORES]
    mesh = Mesh(np.asarray(devices), ("core",))
    in_specs = (PartitionSpec("core"),) * (n_params + n_outs)
    out_specs = (PartitionSpec("core"),) * n_outs
    sharded = jax.jit(
        shard_map(_body, mesh=mesh, in_specs=in_specs, out_specs=out_specs,
                  check_rep=False),
        donate_argnums=donate, keep_unused=True,
    )
    out_sharding = NamedSharding(mesh, PartitionSpec("core"))
    return sharded, zero_outs, out_sharding


_NC = _build_nc()
_SHARDED, _ZERO_OUTS, _OUT_SHARDING = _make_runner(_NC)


def _stage_zeros():
    """Pre-stage the donated output buffer on device (async)."""
    z = np.zeros((N_CORES * _ZERO_OUTS[0].shape[0], *_ZERO_OUTS[0].shape[1:]),
                 _ZERO_OUTS[0].dtype)
    return jax.device_put(z, _OUT_SHARDING)


def _run(inp_concat):
    """Execute the persistent jitted kernel; returns [N_CORES, 10, BS]."""
    global _ZBUF
    out = _SHARDED(inp_concat, _ZBUF)[0]
    _ZBUF = _stage_zeros()   # restage for the next call (donated)
    res = np.asarray(out)
    return res.reshape(N_CORES, 10, BS)


# warm everything once at import: NEFF compile, XLA executable, axon path
_ZBUF = _stage_zeros()
_run(np.zeros((N_CORES * 128, IC), BF16))


def _pack_weights(W1, b1, Wc, bc, Wf, bf, k1):
    """Quantize weights and pack the [128, WW] device weight matrix."""
    qW1, _ = _quant_weight(W1, 3)             # [384, 28]
    qWc, sWc = _quant_weight(Wc, 3)           # [384, 1, 15]
    qWf, sWf = _quant_weight(Wf, 3)           # [10, 28*384]

    # conv block-Toeplitz [128, 5*12*128]:
    #   W_{d,g}[pidx(li,c), pidx(lo,c)] = qWc[c, li - lo + 4d + 7]
    wcb = np.zeros((4, 32, 5, NG, 4, 32), F32)
    qc = qWc[:, 0, :].reshape(NG, 32, KK)     # [g, c, k]
    ci = np.arange(32)
    for dd in range(5):
        for li in range(4):
            for lo in range(4):
                k = li - lo + 4 * (dd - 2) + PAD
                if 0 <= k < KK:
                    wcb[li, ci, dd, :, lo, ci] = qc[:, :, k].T
    # final lhsT [128, 7*12*10]: row pidx(lp,c) of (s,g)-chunk, col j
    wfq = qWf.reshape(10, NS, 4, NG, 32)      # [j, s, lp, g, c]
    wfb = np.ascontiguousarray(wfq.transpose(2, 4, 1, 3, 0))  # [lp,c,s,g,j]
    # stage-B block [rows 0..111], b1/k1 hi/lo rows 112/113, bcr row 114
    w1b = np.zeros((128, NG * 128), F32)
    w1r = w1b[0:112].reshape(4, 28, NG, 4, 32)
    q1g = qW1.reshape(NG, 32, L).transpose(2, 0, 1)    # [f, g, c]
    for lp in range(4):
        w1r[lp, :, :, lp, :] = q1g
    r = (b1 / F32(k1)).astype(F32)                      # [384]
    hi = r.astype(BF16).astype(F32)
    lo = (r - hi).astype(F32)
    w1b[112].reshape(NG, 4, 32)[:] = hi.reshape(NG, 1, 32)
    w1b[113].reshape(NG, 4, 32)[:] = lo.reshape(NG, 1, 32)
    w1b[114].reshape(NG, 4, 32)[:] = bc.reshape(NG, 1, 32)
    wg = np.concatenate(
        [wcb.reshape(128, 5 * NG * 128), wfb.reshape(128, NS * NG * 10), w1b],
        axis=1)
    return wg, sWc, sWf


def kernel(image, W1, b1, Wc, bc, Wf, bf):
    image = np.asarray(image, F32)
    W1 = np.asarray(W1, F32)
    b1 = np.asarray(b1, F32)
    Wc = np.asarray(Wc, F32)
    bc = np.asarray(bc, F32)
    Wf = np.asarray(Wf, F32)
    bf = np.asarray(bf, F32)

    _qW1, sW1 = _quant_weight(W1, 3)
    s0 = _scale(np.abs(image).max(), 8)      # host: exact global input scale
    k1 = float(s0 * sW1)

    wg, sWc, sWf = _pack_weights(W1, b1, Wc, bc, Wf, bf, k1)

    # host input layout: [b, l, f] -> per core [lp, f, lg, b] = [112, 7*512]
    img = image.reshape(N_CORES, BS, NS, 4, L)      # [core, b, lg, lp, f]
    img = img.transpose(0, 3, 4, 2, 1)               # [core, lp, f, lg, b]
    qimg = _rne(img.reshape(N_CORES, 112, QC) / F32(s0))

    # scalar block: fp32 values as bf16 hi/lo pairs, one column each
    sc = np.zeros((128, NSC), F32)
    sc[:, 0] = F32(k1)
    sc[:, 1] = F32(k1 / 127.0)
    sc[:, 2] = F32(sWc)
    sc[:, 3] = F32(sWf)
    # bcp: per-partition bc columns, one per channel-group g
    sc[:, 4:16] = np.broadcast_to(
        bc.reshape(NG, 1, 32).transpose(1, 2, 0), (4, 32, NG)
    ).reshape(128, NG)
    sc[0:10, 16] = bf
    schi = sc.astype(BF16).astype(F32)
    sclo = sc - schi

    inp = np.zeros((N_CORES, 128, IC), BF16)
    inp[:, 0:112, 0:QC] = qimg.astype(BF16)
    inp[:, 112:114, 0:QC] = BF16(1.0)                # stage-B bias ones rows
    inp[:, :, WCH0:SC0] = (
        wg.astype(BF16).reshape(N_CORES, 128, WCHW))
    inp[:, :, SC0:SC0 + NSC] = schi.astype(BF16)
    inp[:, :, SC0 + NSC:IC] = sclo.astype(BF16)

    res = _run(np.ascontiguousarray(inp.reshape(N_CORES * 128, IC)))

    # gather + host-side exact final fake-quant (s5 global)
    logits = res.transpose(0, 2, 1).reshape(BATCH, 10)
    s5 = _scale(np.abs(logits).max(), 8)
    out = (_rne(logits / s5) * s5).astype(F32)

    class _R:
        exec_time_ns = None
    kernel.last_results = _R()
    return out


# revision 9
# speedup vs baseline: 7.8214x; 1.1177x over previous
"""nn_ConvModel — Bass/Tile kernel, data-parallel over 8 TRN2 NeuronCores.

Strategy (per sharding_hint): batch dim of `image` sharded 8 ways, tiny
3-bit-quantized weights replicated on device via an in-kernel AllGather
(each core ships only a 1/8 row-chunk of the packed weight matrix, so the
slow host->device axon link carries each weight byte once).  The two
data-dependent activation quant scales (s1 for lin, s3 for the depthwise
conv output) are computed on-device as shard-local abs-maxes +
AllReduce(max).  The input scale s0 (pure function of the input) is
applied on the host: the image ships pre-quantized as bf16 integers.  The
final logits scale s5 (160 KB tensor) is applied on the host, exactly.

All input-dependent scalars (k1 = s0*sW1, sWc, sWf, biases) enter the
device as tensors (bf16 hi/lo pairs reassembled to fp32 on device), so
the Bass program itself is input-value-independent: it is traced,
compiled and warm-executed once at import time, and kernel() only packs
inputs, runs the persistent jitted executable, and unpacks.

Device layout (per core, batch shard b=512):
  partitions = (l%4)*32 + channel%32   [l = sequence pos 0..27, 28=7*4]
  free       = batch
  * stage-B linear:  lhsT[(f28,lp4)+ones=113, (l4,c32)=128] block-diag in
    l with the bias row b1/k1 folded in; one matmul per (channel-group g
    of 12, l-slab s of 7), N=512.
  * depthwise conv:  block-Toeplitz 128x128 weights W_d (d=-2..2), ~29
    accumulated TensorE matmuls per (g,s); no transposes anywhere.
  * final linear:    Wf rearranged to the same (l4,c32) partition order,
    84 accumulating matmuls into one [10,512] PSUM tile.
  * fake-quant rounding = (+1.5*2^23, -1.5*2^23) round-to-nearest-even,
    spread across ScalarE, VectorE and GpSimd; tanh on ScalarE.
Three phases (PSUM cannot hold lin, SBUF cannot hold it in fp32):
  PH1 stage-B matmuls + abs-max from PSUM -> AllReduce(max) -> s1
  PH2 stage-B recompute -> q1 (bf16 ints, resident) -> conv (+bias row
      matmul) -> abs-max -> AllReduce(max) -> s3
  PH3 conv recompute -> q2 -> final matmul -> logits out.
"""
import sys
import os as _os
import numpy as np

sys.path.insert(0, "/opt/trn_rl_repo")

import ml_dtypes  # noqa: E402

try:
    import antenv.axon_hooks  # noqa: F401,E402
except ImportError:
    # axon NTFF profiling hook unavailable here: a trace request would
    # crash inside the axon run path, so force tracing off.
    _os.environ["BASS_NEVER_TRACE"] = "1"
import jax  # noqa: E402
try:
    jax.config.update("jax_compilation_cache_dir", "/tmp/jax_pcache")
    jax.config.update("jax_persistent_cache_min_compile_time_secs", 0.0)
    jax.config.update("jax_persistent_cache_min_entry_size_bytes", -1)
except Exception:
    pass
jax.devices()  # initialize the PJRT client eagerly

import concourse.bacc as bacc  # noqa: E402
import concourse.tile as tile  # noqa: E402
import concourse.mybir as mybir  # noqa: E402
from concourse import bass2jax  # noqa: E402
from jax.experimental.shard_map import shard_map  # noqa: E402
from jax.sharding import Mesh, PartitionSpec, NamedSharding  # noqa: E402

N_CORES = 8
BATCH = 4096
BS = BATCH // N_CORES          # 512 per-core batch shard
MD = 384                        # model dim / channels
KK = 15                         # conv kernel taps
PAD = 7
L = 28                          # sequence length
NG = 12                         # channel groups of 32
NS = 7                          # l-slabs of 4
F32 = np.float32
BF16 = ml_dtypes.bfloat16

_M = F32(12582912.0)            # 1.5 * 2^23 : (x+M)-M == round-half-even(x)

# single bf16 input tensor [128, IC] per core:
QC = NS * BS                    # 3584  qx cols (rows 0..113 used)
WCH0 = QC                       # weight-chunk cols (1/8 of wg, reshaped)
WW = 5 * NG * 128 + NS * NG * 10 + NG * 128   # 7680 + 840 + 1536 = 10056
WCHW = WW * 16 // 128           # 1257
SC0 = WCH0 + WCHW               # 4841  scalar hi/lo cols
NSC = 17                        # k1, k1/127, sWc, sWf, bcp[12], bfp
IC = SC0 + 2 * NSC              # 4875
# wg (gathered weights) column layout
WCOFF = 0                       # conv block-Toeplitz  [128, 7680]
WFOFF = 5 * NG * 128            # final linear         [128, 840]
W1OFF = WFOFF + NS * NG * 10    # stage-B block + bias rows + bcr [115, 1536]


def _rne(x):
    return (x.astype(F32) + _M) - _M


def _scale(absmax, bits):
    qmax = F32(2 ** (bits - 1) - 1)
    return np.maximum(F32(absmax) / qmax, F32(1e-8))


def _quant_weight(w, bits):
    s = _scale(np.abs(w).max(), bits)
    q = _rne(w / s).astype(F32)
    return q, s


def _build_nc():
    """Trace the input-value-independent Bass/Tile kernel."""
    dt = mybir.dt
    ALU = mybir.AluOpType
    AFT = mybir.ActivationFunctionType
    AXL = mybir.AxisListType

    nc = bacc.Bacc("TRN2", target_bir_lowering=False, debug=False,
                   num_devices=N_CORES)

    inp_d = nc.dram_tensor("inp", [128, IC], dt.bfloat16,
                           kind="ExternalInput")
    # all-gathered logits: every core holds the full [8*10, BS] result, so
    # the host fetches from a single device (one RPC instead of eight).
    out_d = nc.dram_tensor("out", [N_CORES * 10, BS], dt.float32,
                           kind="ExternalOutput")

    rg = [list(range(N_CORES))]

    with tile.TileContext(nc) as tc:
        with (
            tc.tile_pool(name="const", bufs=1) as cpool,
            tc.tile_pool(name="work", bufs=2) as wpool,
            tc.tile_pool(name="scal", bufs=1) as spool,
            tc.tile_pool(name="ps1", bufs=2, space="PSUM") as ps1,
            tc.tile_pool(name="ps3", bufs=2, space="PSUM") as ps3,
            tc.tile_pool(name="psf", bufs=1, space="PSUM") as psf,
            tc.tile_pool(name="psb", bufs=1, space="PSUM") as psb,
            tc.tile_pool(name="dram", bufs=1, space="DRAM") as dpool,
        ):
            # ---- weight AllGather: each core contributed rows 16r..16r+16
            # of the packed [128, WW] weight matrix as a [128, WCHW] blob.
            # Collective inputs must be contiguous in DRAM: stage the
            # strided column block through a contiguous DRAM tile first.
            wch_t = dpool.tile([128, WCHW], dt.bfloat16)
            nc.sync.dma_start(wch_t, inp_d[:, WCH0:WCH0 + WCHW])
            wg_t = dpool.tile([128, WW], dt.bfloat16, addr_space="Shared")
            nc.gpsimd.collective_compute(
                "AllGather", ALU.bypass,
                ins=[wch_t.opt()], outs=[wg_t.opt()],
                replica_groups=rg)

            # ---- per-core inputs straight from DRAM
            qx = cpool.tile([114, QC], dt.bfloat16)
            nc.sync.dma_start(qx, inp_d[0:114, 0:QC])
            hl_t = cpool.tile([128, 2 * NSC], dt.bfloat16)
            nc.sync.dma_start(hl_t, inp_d[:, SC0:SC0 + 2 * NSC])
            sc_t = cpool.tile([128, NSC], dt.float32)
            nc.vector.tensor_add(sc_t, hl_t[:, 0:NSC], hl_t[:, NSC:2 * NSC])
            k1_c = sc_t[:, 0:1]
            k1d127_c = sc_t[:, 1:2]
            sWc_c = sc_t[:, 2:3]
            sWf_c = sc_t[:, 3:4]
            bc_t = sc_t[:, 4:16]
            bf_c = sc_t[:, 16:17]

            # ---- gathered weights into SBUF
            wc_t = cpool.tile([128, 5 * NG * 128], dt.bfloat16)
            nc.sync.dma_start(wc_t, wg_t[:, WCOFF:WCOFF + 5 * NG * 128])
            wf_t = cpool.tile([128, NS * NG * 10], dt.bfloat16)
            nc.sync.dma_start(wf_t, wg_t[:, WFOFF:WFOFF + NS * NG * 10])
            w1_t = cpool.tile([114, NG * 128], dt.bfloat16)
            nc.sync.dma_start(w1_t, wg_t[0:114, W1OFF:W1OFF + NG * 128])
            bcr_t = cpool.tile([1, NG * 128], dt.bfloat16)
            nc.sync.dma_start(bcr_t, wg_t[114:115, W1OFF:W1OFF + NG * 128])

            ones_r = cpool.tile([1, 128], dt.float32)     # bcast lhsT
            ones_b = cpool.tile([1, BS], dt.float32)      # bias-mm rhs helper
            nc.gpsimd.memset(ones_r, 1.0)
            nc.gpsimd.memset(ones_b, 1.0)
            mM_t = cpool.tile([128, 1], dt.float32)
            nc.gpsimd.memset(mM_t, float(_M))

            q1_t = cpool.tile([128, NG * NS * BS], dt.bfloat16)
            mbuf = spool.tile([128, NG * NS], dt.float32)
            m3buf = spool.tile([128, NG * NS], dt.float32)

            def stage_b_mm(g, s):
                p = ps1.tile([128, BS], dt.float32, tag="ps1", name=f"p1_{g}_{s}")
                nc.tensor.matmul(p, w1_t[0:114, g * 128:(g + 1) * 128],
                                 qx[0:114, s * BS:(s + 1) * BS],
                                 start=True, stop=True)
                return p

            def conv_mm(g, s, bias_rhs=None):
                p3 = ps3.tile([128, BS], dt.float32, tag="ps3",
                              name=f"p3_{g}_{s}")
                dmin = max(-2, -s)
                dmax = min(2, (NS - 1) - s)
                for d in range(dmin, dmax + 1):
                    col0 = ((d + 2) * NG + g) * 128
                    nc.tensor.matmul(
                        p3, wc_t[:, col0:col0 + 128],
                        q1_t[:, (g * NS + s + d) * BS:(g * NS + s + d + 1) * BS],
                        start=(d == dmin), stop=(d == dmax and bias_rhs is None))
                if bias_rhs is not None:
                    nc.tensor.matmul(p3, bcr_t[0:1, g * 128:(g + 1) * 128],
                                     bias_rhs, start=False, stop=True)
                return p3

            # ---------------- PH1: abs-max of stage-B psum -----------------
            for g in range(NG):
                for s in range(NS):
                    p = stage_b_mm(g, s)
                    nc.vector.tensor_reduce(
                        mbuf[:, g * NS + s: g * NS + s + 1], p, axis=AXL.X,
                        op=ALU.max, apply_absolute_value=True)

            mred = spool.tile([128, 1], dt.float32)
            nc.vector.tensor_reduce(mred, mbuf, axis=AXL.X, op=ALU.max)
            m1s = spool.tile([1, 8], dt.float32)
            nc.gpsimd.memset(m1s, 0.0)
            nc.gpsimd.tensor_reduce(m1s[0:1, 0:1], mred, axis=AXL.C, op=ALU.max)

            ar_in1 = dpool.tile([1, 8], dt.float32)
            ar_out1 = dpool.tile([1, 8], dt.float32, addr_space="Shared")
            nc.sync.dma_start(ar_in1, m1s)
            nc.gpsimd.collective_compute(
                "AllReduce", ALU.max, ins=[ar_in1.opt()], outs=[ar_out1.opt()],
                replica_groups=rg)
            m1g = spool.tile([1, 8], dt.float32)
            nc.sync.dma_start(m1g, ar_out1[:])

            # broadcast global max to [128,1] via ones-lhsT matmul
            pb = psb.tile([128, 1], dt.float32, tag="pb", name="pb1")
            nc.tensor.matmul(pb, ones_r, m1g[0:1, 0:1], start=True, stop=True)
            m1t = spool.tile([128, 1], dt.float32)
            nc.scalar.activation(m1t, pb, AFT.Copy)

            # scalar chain 1 (m1t = max|raw+b1/k1| -> s1 = max(m*k1/127,1e-8))
            s1_t = spool.tile([128, 1], dt.float32)
            nc.vector.tensor_mul(s1_t, m1t, k1d127_c)
            nc.vector.tensor_scalar(s1_t, s1_t, float(1e-8), None, ALU.max)
            inv_s1 = spool.tile([128, 1], dt.float32)
            nc.vector.reciprocal(inv_s1, s1_t)
            a1_t = spool.tile([128, 1], dt.float32)
            nc.vector.tensor_mul(a1_t, inv_s1, k1_c)
            th1 = spool.tile([128, 1], dt.float32)
            nc.scalar.activation(th1, s1_t, AFT.Tanh, scale=127.0)
            s2_t = spool.tile([128, 1], dt.float32)
            nc.vector.tensor_scalar(s2_t, th1, float(1.0 / 127.0), float(1e-8),
                                    ALU.mult, ALU.max)
            inv_s2 = spool.tile([128, 1], dt.float32)
            nc.vector.reciprocal(inv_s2, s2_t)
            k3_t = spool.tile([128, 1], dt.float32)
            nc.vector.tensor_mul(k3_t, s2_t, sWc_c)
            inv_k3 = spool.tile([128, 1], dt.float32)
            nc.vector.reciprocal(inv_k3, k3_t)
            # device row [1, BS] of 1/k3 for the conv bias matmul (bf16)
            rk3_f = spool.tile([1, BS], dt.float32)
            nc.vector.scalar_tensor_tensor(rk3_f, ones_b, inv_k3[0:1, 0:1],
                                           ones_b, ALU.mult, ALU.mult)
            rk3 = spool.tile([1, BS], dt.bfloat16)
            nc.vector.tensor_copy(rk3, rk3_f)

            def quant_chain(p, a_ap, bias_ap, sc_ap, invn_ap, qdst, nm):
                """qdst (bf16 ints) = rne(tanh(sc*rne(p*a + bias)) * invn).

                bias_ap may be None when the bias is already inside p (then
                the +M is fused into the ScalarE affine drain)."""
                w = wpool.tile([128, BS], dt.float32, tag="ew", name=f"w{nm}")
                if bias_ap is None:
                    nc.scalar.activation(w, p, AFT.Identity, bias=mM_t,
                                         scale=a_ap)
                    ql = wpool.tile([128, BS], dt.bfloat16, tag="eql",
                                    name=f"ql{nm}")
                    nc.gpsimd.tensor_scalar(ql, w, float(-_M), None, ALU.add)
                else:
                    nc.scalar.activation(w, p, AFT.Identity, bias=bias_ap,
                                         scale=a_ap)
                    ql = wpool.tile([128, BS], dt.bfloat16, tag="eql",
                                    name=f"ql{nm}")
                    nc.vector.tensor_scalar(ql, w, float(_M), float(-_M),
                                            ALU.add, ALU.add)
                t = wpool.tile([128, BS], dt.float32, tag="et", name=f"t{nm}")
                nc.scalar.activation(t, ql, AFT.Tanh, scale=sc_ap)
                v = wpool.tile([128, BS], dt.float32, tag="ev", name=f"v{nm}")
                nc.vector.tensor_scalar(v, t, invn_ap, float(_M),
                                        ALU.mult, ALU.add)
                nc.gpsimd.tensor_scalar(qdst, v, float(-_M), None, ALU.add)

            # ---------------- PH2: q1, conv(+bias), abs-max ----------------
            for g in range(NG):
                for s in range(NS):
                    p = stage_b_mm(g, s)
                    quant_chain(p, a1_t, None, s1_t, inv_s2,
                                q1_t[:, (g * NS + s) * BS:(g * NS + s + 1) * BS],
                                f"b{g}_{s}")
            for g in range(NG):
                for s in range(NS):
                    p3 = conv_mm(g, s, bias_rhs=rk3)
                    nc.vector.tensor_reduce(
                        m3buf[:, g * NS + s: g * NS + s + 1], p3, axis=AXL.X,
                        op=ALU.max, apply_absolute_value=True)

            m3red = spool.tile([128, 1], dt.float32)
            nc.vector.tensor_reduce(m3red, m3buf, axis=AXL.X, op=ALU.max)
            m3s = spool.tile([1, 8], dt.float32)
            nc.gpsimd.memset(m3s, 0.0)
            nc.gpsimd.tensor_reduce(m3s[0:1, 0:1], m3red, axis=AXL.C, op=ALU.max)

            ar_in2 = dpool.tile([1, 8], dt.float32)
            ar_out2 = dpool.tile([1, 8], dt.float32, addr_space="Shared")
            nc.sync.dma_start(ar_in2, m3s)
            nc.gpsimd.collective_compute(
                "AllReduce", ALU.max, ins=[ar_in2.opt()], outs=[ar_out2.opt()],
                replica_groups=rg)
            m3g = spool.tile([1, 8], dt.float32)
            nc.sync.dma_start(m3g, ar_out2[:])
            pb3 = psb.tile([128, 1], dt.float32, tag="pb", name="pb3")
            nc.tensor.matmul(pb3, ones_r, m3g[0:1, 0:1], start=True, stop=True)
            m3t = spool.tile([128, 1], dt.float32)
            nc.scalar.activation(m3t, pb3, AFT.Copy)

            # scalar chain 2: m3 = max|raw3+bc/k3| -> s3 = max(m3*k3/127,1e-8)
            s3_t = spool.tile([128, 1], dt.float32)
            nc.vector.tensor_mul(s3_t, m3t, k3_t)
            nc.vector.tensor_scalar(s3_t, s3_t, float(1.0 / 127.0), float(1e-8),
                                    ALU.mult, ALU.max)
            inv_s3 = spool.tile([128, 1], dt.float32)
            nc.vector.reciprocal(inv_s3, s3_t)
            a3_t = spool.tile([128, 1], dt.float32)
            nc.vector.tensor_mul(a3_t, k3_t, inv_s3)
            th3 = spool.tile([128, 1], dt.float32)
            nc.scalar.activation(th3, s3_t, AFT.Tanh, scale=127.0)
            s4_t = spool.tile([128, 1], dt.float32)
            nc.vector.tensor_scalar(s4_t, th3, float(1.0 / 127.0), float(1e-8),
                                    ALU.mult, ALU.max)
            inv_s4 = spool.tile([128, 1], dt.float32)
            nc.vector.reciprocal(inv_s4, s4_t)
            k5_t = spool.tile([128, 1], dt.float32)
            nc.vector.tensor_mul(k5_t, s4_t, sWf_c)
            bcs3 = spool.tile([128, NG], dt.float32)
            for g in range(NG):
                nc.vector.tensor_mul(bcs3[:, g:g + 1], bc_t[:, g:g + 1], inv_s3)

            # ---------------- PH3: conv recompute, q2, final ---------------
            pf = psf.tile([10, BS], dt.float32)
            n_acc = NG * NS
            idx = 0
            for g in range(NG):
                for s in range(NS):
                    p3 = conv_mm(g, s)
                    q2 = wpool.tile([128, BS], dt.bfloat16, tag="q2",
                                    name=f"q2_{g}_{s}")
                    quant_chain(p3, a3_t, bcs3[:, g:g + 1], s3_t, inv_s4, q2,
                                f"d{g}_{s}")
                    col0 = (s * NG + g) * 10
                    nc.tensor.matmul(pf, wf_t[:, col0:col0 + 10], q2,
                                     start=(idx == 0), stop=(idx == n_acc - 1),
                                     skip_group_check=True)
                    idx += 1

            lg_sb = wpool.tile([10, BS], dt.float32, tag="lg")
            nc.vector.tensor_scalar(lg_sb, pf, k5_t[0:10, 0:1],
                                    bf_c[0:10, 0:1], ALU.mult, ALU.add)
            lg_d = dpool.tile([10, BS], dt.float32)
            nc.sync.dma_start(lg_d, lg_sb)
            lg_all = dpool.tile([N_CORES * 10, BS], dt.float32,
                                addr_space="Shared")
            nc.gpsimd.collective_compute(
                "AllGather", ALU.bypass, ins=[lg_d.opt()], outs=[lg_all.opt()],
                replica_groups=rg)
            nc.sync.dma_start(out_d[:], lg_all[:])

    nc.compile()
    return nc


def _make_runner(nc):
    """Persistent jitted SPMD executable over 8 cores.

    Mirrors concourse.bass2jax.run_bass_via_pjrt's multi-core path, but
    the jit (and hence the traced/lowered/compiled executable) is built
    once at import and reused on every kernel() call.
    """
    bass2jax.install_neuronx_cc_hook()
    partition_name = (nc.partition_id_tensor.name
                      if nc.partition_id_tensor else None)

    in_names, out_names, out_avals = [], [], []
    for alloc in nc.m.functions[0].allocations:
        if not isinstance(alloc, mybir.MemoryLocationSet):
            continue
        name = alloc.memorylocations[0].name
        if alloc.kind == "ExternalInput":
            if name != partition_name:
                in_names.append(name)
        elif alloc.kind == "ExternalOutput":
            shape = tuple(alloc.tensor_shape)
            dtype = mybir.dt.np(alloc.dtype)
            out_names.append(name)
            out_avals.append(jax.core.ShapedArray(shape, dtype))
    n_params = len(in_names)
    # The kernel writes every element of the output, so no donated
    # pre-zeroed output operands are needed (upstream run_bass_via_pjrt
    # threads them only for kernels with partially-written outputs).
    if partition_name is not None:
        in_names.append(partition_name)

    def _body(*args):
        operands = list(args)
        if partition_name is not None:
            operands.append(bass2jax.partition_id_tensor())
        outs = bass2jax._bass_exec_p.bind(
            *operands,
            out_avals=tuple(out_avals),
            in_names=tuple(in_names),
            out_names=tuple(out_names),
            lowering_input_output_aliases=(),
            sim_require_finite=True,
            sim_require_nnan=True,
            nc=nc,
        )
        return tuple(outs)

    devices = jax.devices()[:N_CORES]
    mesh = Mesh(np.asarray(devices), ("core",))
    in_specs = (PartitionSpec("core"),) * n_params
    # the kernel all-gathers logits, so every core returns the identical
    # full [8*10, BS] tensor -> replicated out_spec, single-shard fetch.
    out_specs = (PartitionSpec(),) * len(out_names)
    sharded = jax.jit(
        shard_map(_body, mesh=mesh, in_specs=in_specs, out_specs=out_specs,
                  check_rep=False),
        keep_unused=True,
    )
    return sharded


_NC = _build_nc()
_SHARDED = _make_runner(_NC)


def _run(inp_concat):
    """Execute the persistent jitted kernel; returns [N_CORES, 10, BS]."""
    out = _SHARDED(inp_concat)[0]
    res = np.asarray(out)
    return res.reshape(N_CORES, 10, BS)


# warm everything once at import: NEFF compile, XLA executable, axon path
_run(np.zeros((N_CORES * 128, IC), BF16))


def _pack_weights(W1, b1, Wc, bc, Wf, bf, k1):
    """Quantize weights and pack the [128, WW] device weight matrix."""
    qW1, _ = _quant_weight(W1, 3)             # [384, 28]
    qWc, sWc = _quant_weight(Wc, 3)           # [384, 1, 15]
    qWf, sWf = _quant_weight(Wf, 3)           # [10, 28*384]

    # conv block-Toeplitz [128, 5*12*128]:
    #   W_{d,g}[pidx(li,c), pidx(lo,c)] = qWc[c, li - lo + 4d + 7]
    wcb = np.zeros((4, 32, 5, NG, 4, 32), F32)
    qc = qWc[:, 0, :].reshape(NG, 32, KK)     # [g, c, k]
    ci = np.arange(32)
    for dd in range(5):
        for li in range(4):
            for lo in range(4):
                k = li - lo + 4 * (dd - 2) + PAD
                if 0 <= k < KK:
                    wcb[li, ci, dd, :, lo, ci] = qc[:, :, k].T
    # final lhsT [128, 7*12*10]: row pidx(lp,c) of (s,g)-chunk, col j
    wfq = qWf.reshape(10, NS, 4, NG, 32)      # [j, s, lp, g, c]
    wfb = np.ascontiguousarray(wfq.transpose(2, 4, 1, 3, 0))  # [lp,c,s,g,j]
    # stage-B block [rows 0..111], b1/k1 hi/lo rows 112/113, bcr row 114
    w1b = np.zeros((128, NG * 128), F32)
    w1r = w1b[0:112].reshape(4, 28, NG, 4, 32)
    q1g = qW1.reshape(NG, 32, L).transpose(2, 0, 1)    # [f, g, c]
    for lp in range(4):
        w1r[lp, :, :, lp, :] = q1g
    r = (b1 / F32(k1)).astype(F32)                      # [384]
    hi = r.astype(BF16).astype(F32)
    lo = (r - hi).astype(F32)
    w1b[112].reshape(NG, 4, 32)[:] = hi.reshape(NG, 1, 32)
    w1b[113].reshape(NG, 4, 32)[:] = lo.reshape(NG, 1, 32)
    w1b[114].reshape(NG, 4, 32)[:] = bc.reshape(NG, 1, 32)
    wg = np.concatenate(
        [wcb.reshape(128, 5 * NG * 128), wfb.reshape(128, NS * NG * 10), w1b],
        axis=1)
    return wg, sWc, sWf


def kernel(image, W1, b1, Wc, bc, Wf, bf):
    image = np.asarray(image, F32)
    W1 = np.asarray(W1, F32)
    b1 = np.asarray(b1, F32)
    Wc = np.asarray(Wc, F32)
    bc = np.asarray(bc, F32)
    Wf = np.asarray(Wf, F32)
    bf = np.asarray(bf, F32)

    _qW1, sW1 = _quant_weight(W1, 3)
    s0 = _scale(np.abs(image).max(), 8)      # host: exact global input scale
    k1 = float(s0 * sW1)

    wg, sWc, sWf = _pack_weights(W1, b1, Wc, bc, Wf, bf, k1)

    # host input layout: [b, l, f] -> per core [lp, f, lg, b] = [112, 7*512]
    img = image.reshape(N_CORES, BS, NS, 4, L)      # [core, b, lg, lp, f]
    img = img.transpose(0, 3, 4, 2, 1)               # [core, lp, f, lg, b]
    qimg = _rne(img.reshape(N_CORES, 112, QC) / F32(s0))

    # scalar block: fp32 values as bf16 hi/lo pairs, one column each
    sc = np.zeros((128, NSC), F32)
    sc[:, 0] = F32(k1)
    sc[:, 1] = F32(k1 / 127.0)
    sc[:, 2] = F32(sWc)
    sc[:, 3] = F32(sWf)
    # bcp: per-partition bc columns, one per channel-group g
    sc[:, 4:16] = np.broadcast_to(
        bc.reshape(NG, 1, 32).transpose(1, 2, 0), (4, 32, NG)
    ).reshape(128, NG)
    sc[0:10, 16] = bf
    schi = sc.astype(BF16).astype(F32)
    sclo = sc - schi

    inp = np.zeros((N_CORES, 128, IC), BF16)
    inp[:, 0:112, 0:QC] = qimg.astype(BF16)
    inp[:, 112:114, 0:QC] = BF16(1.0)                # stage-B bias ones rows
    inp[:, :, WCH0:SC0] = (
        wg.astype(BF16).reshape(N_CORES, 128, WCHW))
    inp[:, :, SC0:SC0 + NSC] = schi.astype(BF16)
    inp[:, :, SC0 + NSC:IC] = sclo.astype(BF16)

    res = _run(np.ascontiguousarray(inp.reshape(N_CORES * 128, IC)))

    # gather + host-side exact final fake-quant (s5 global)
    logits = res.transpose(0, 2, 1).reshape(BATCH, 10)
    s5 = _scale(np.abs(logits).max(), 8)
    out = (_rne(logits / s5) * s5).astype(F32)

    class _R:
        exec_time_ns = None
    kernel.last_results = _R()
    return out


# revision 12
# speedup vs baseline: 10.2938x; 1.3161x over previous
"""nn_ConvModel — Bass/Tile kernel, data-parallel over 8 TRN2 NeuronCores.

Strategy (per sharding_hint): batch dim of `image` sharded 8 ways, tiny
3-bit-quantized weights replicated on device via an in-kernel AllGather
(each core ships only a 1/8 row-chunk of the packed weight matrix, so the
slow host->device axon link carries each weight byte once).  The two
data-dependent activation quant scales (s1 for lin, s3 for the depthwise
conv output) are computed on-device as shard-local abs-maxes +
AllReduce(max).  The input scale s0 (pure function of the input) is
applied on the host: the image ships pre-quantized as bf16 integers.  The
final logits scale s5 (160 KB tensor) is applied on the host, exactly.

All input-dependent scalars (k1 = s0*sW1, sWc, sWf, biases) enter the
device as tensors (bf16 hi/lo pairs reassembled to fp32 on device), so
the Bass program itself is input-value-independent: it is traced,
compiled and warm-executed once at import time, and kernel() only packs
inputs, runs the persistent jitted executable, and unpacks.

Device layout (per core, batch shard b=512):
  partitions = (l%4)*32 + channel%32   [l = sequence pos 0..27, 28=7*4]
  free       = batch
  * stage-B linear:  lhsT[(f28,lp4)+ones=113, (l4,c32)=128] block-diag in
    l with the bias row b1/k1 folded in; one matmul per (channel-group g
    of 12, l-slab s of 7), N=512.
  * depthwise conv:  block-Toeplitz 128x128 weights W_d (d=-2..2), ~29
    accumulated TensorE matmuls per (g,s); no transposes anywhere.
  * final linear:    Wf rearranged to the same (l4,c32) partition order,
    84 accumulating matmuls into one [10,512] PSUM tile.
  * fake-quant rounding = (+1.5*2^23, -1.5*2^23) round-to-nearest-even,
    spread across ScalarE, VectorE and GpSimd; tanh on ScalarE.
Three phases (PSUM cannot hold lin, SBUF cannot hold it in fp32):
  PH1 stage-B matmuls + abs-max from PSUM -> AllReduce(max) -> s1
  PH2 stage-B recompute -> q1 (bf16 ints, resident) -> conv (+bias row
      matmul) -> abs-max -> AllReduce(max) -> s3
  PH3 conv recompute -> q2 -> final matmul -> logits out.
"""
import sys
import os as _os
import numpy as np

sys.path.insert(0, "/opt/trn_rl_repo")

import ml_dtypes  # noqa: E402

try:
    import antenv.axon_hooks  # noqa: F401,E402
except ImportError:
    # axon NTFF profiling hook unavailable here: a trace request would
    # crash inside the axon run path, so force tracing off.
    _os.environ["BASS_NEVER_TRACE"] = "1"
import jax  # noqa: E402
try:
    jax.config.update("jax_compilation_cache_dir", "/tmp/jax_pcache")
    jax.config.update("jax_persistent_cache_min_compile_time_secs", 0.0)
    jax.config.update("jax_persistent_cache_min_entry_size_bytes", -1)
except Exception:
    pass
jax.devices()  # initialize the PJRT client eagerly

import concourse.bacc as bacc  # noqa: E402
import concourse.tile as tile  # noqa: E402
import concourse.mybir as mybir  # noqa: E402
from concourse import bass2jax  # noqa: E402
from jax.experimental.shard_map import shard_map  # noqa: E402
from jax.sharding import Mesh, PartitionSpec, NamedSharding  # noqa: E402

N_CORES = 8
BATCH = 4096
BS = BATCH // N_CORES          # 512 per-core batch shard
MD = 384                        # model dim / channels
KK = 15                         # conv kernel taps
PAD = 7
L = 28                          # sequence length
NG = 12                         # channel groups of 32
NS = 7                          # l-slabs of 4
F32 = np.float32
BF16 = ml_dtypes.bfloat16

_M = F32(12582912.0)            # 1.5 * 2^23 : (x+M)-M == round-half-even(x)

# single bf16 input tensor [128, IC] per core:
QC = NS * BS                    # 3584  qx ints (int8, rows 0..113 used)
QCB = QC // 2                   # 1792  bf16 cols holding the int8 qx bytes
WCH0 = QCB                      # weight-chunk cols (1/8 of wg, reshaped)
WW = 5 * NG * 128 + NS * NG * 10 + NG * 128   # 7680 + 840 + 1536 = 10056
WCHW = WW * 16 // 128           # 1257
SC0 = WCH0 + WCHW               # 3049  scalar hi/lo cols
NSC = 17                        # k1, k1/127, sWc, sWf, bcp[12], bfp
IC = SC0 + 2 * NSC              # 3083
# wg (gathered weights) column layout
WCOFF = 0                       # conv block-Toeplitz  [128, 7680]
WFOFF = 5 * NG * 128            # final linear         [128, 840]
W1OFF = WFOFF + NS * NG * 10    # stage-B block + bias rows + bcr [115, 1536]


def _rne(x):
    return (x.astype(F32) + _M) - _M


def _scale(absmax, bits):
    qmax = F32(2 ** (bits - 1) - 1)
    return np.maximum(F32(absmax) / qmax, F32(1e-8))


def _quant_weight(w, bits):
    s = _scale(np.abs(w).max(), bits)
    q = _rne(w / s).astype(F32)
    return q, s


def _build_nc():
    """Trace the input-value-independent Bass/Tile kernel."""
    dt = mybir.dt
    ALU = mybir.AluOpType
    AFT = mybir.ActivationFunctionType
    AXL = mybir.AxisListType

    nc = bacc.Bacc("TRN2", target_bir_lowering=False, debug=False,
                   num_devices=N_CORES)

    inp_d = nc.dram_tensor("inp", [128, IC], dt.bfloat16,
                           kind="ExternalInput")
    # all-gathered logits: every core holds the full [8*10, BS] result, so
    # the host fetches from a single device (one RPC instead of eight).
    out_d = nc.dram_tensor("out", [N_CORES * 10, BS], dt.float32,
                           kind="ExternalOutput")

    rg = [list(range(N_CORES))]

    with tile.TileContext(nc) as tc:
        with (
            tc.tile_pool(name="const", bufs=1) as cpool,
            tc.tile_pool(name="work", bufs=2) as wpool,
            tc.tile_pool(name="scal", bufs=1) as spool,
            tc.tile_pool(name="ps1", bufs=2, space="PSUM") as ps1,
            tc.tile_pool(name="ps3", bufs=2, space="PSUM") as ps3,
            tc.tile_pool(name="psf", bufs=1, space="PSUM") as psf,
            tc.tile_pool(name="psb", bufs=1, space="PSUM") as psb,
            tc.tile_pool(name="dram", bufs=1, space="DRAM") as dpool,
        ):
            # ---- weight AllGather: each core contributed rows 16r..16r+16
            # of the packed [128, WW] weight matrix as a [128, WCHW] blob.
            # Collective inputs must be contiguous in DRAM: stage the
            # strided column block through a contiguous DRAM tile first.
            wch_t = dpool.tile([128, WCHW], dt.bfloat16)
            nc.sync.dma_start(wch_t, inp_d[:, WCH0:WCH0 + WCHW])
            wg_t = dpool.tile([128, WW], dt.bfloat16, addr_space="Shared")
            nc.gpsimd.collective_compute(
                "AllGather", ALU.bypass,
                ins=[wch_t.opt()], outs=[wg_t.opt()],
                replica_groups=rg)

            # ---- per-core inputs straight from DRAM
            # qx ships as int8 (exact for [-128,127]); convert to bf16 on
            # VectorE (exact, verified) to halve the host->device bytes.
            q8 = cpool.tile([114, QC], dt.int8)
            nc.sync.dma_start(q8, inp_d[0:114, 0:QCB].bitcast(dt.int8))
            qx = cpool.tile([114, QC], dt.bfloat16)
            nc.vector.tensor_copy(qx, q8)
            hl_t = cpool.tile([128, 2 * NSC], dt.bfloat16)
            nc.sync.dma_start(hl_t, inp_d[:, SC0:SC0 + 2 * NSC])
            sc_t = cpool.tile([128, NSC], dt.float32)
            nc.vector.tensor_add(sc_t, hl_t[:, 0:NSC], hl_t[:, NSC:2 * NSC])
            k1_c = sc_t[:, 0:1]
            k1d127_c = sc_t[:, 1:2]
            sWc_c = sc_t[:, 2:3]
            sWf_c = sc_t[:, 3:4]
            bc_t = sc_t[:, 4:16]
            bf_c = sc_t[:, 16:17]

            # ---- gathered weights into SBUF
            wc_t = cpool.tile([128, 5 * NG * 128], dt.bfloat16)
            nc.sync.dma_start(wc_t, wg_t[:, WCOFF:WCOFF + 5 * NG * 128])
            wf_t = cpool.tile([128, NS * NG * 10], dt.bfloat16)
            nc.sync.dma_start(wf_t, wg_t[:, WFOFF:WFOFF + NS * NG * 10])
            w1_t = cpool.tile([114, NG * 128], dt.bfloat16)
            nc.sync.dma_start(w1_t, wg_t[0:114, W1OFF:W1OFF + NG * 128])
            bcr_t = cpool.tile([1, NG * 128], dt.bfloat16)
            nc.sync.dma_start(bcr_t, wg_t[114:115, W1OFF:W1OFF + NG * 128])

            ones_r = cpool.tile([1, 128], dt.float32)     # bcast lhsT
            ones_b = cpool.tile([1, BS], dt.float32)      # bias-mm rhs helper
            nc.gpsimd.memset(ones_r, 1.0)
            nc.gpsimd.memset(ones_b, 1.0)
            mM_t = cpool.tile([128, 1], dt.float32)
            nc.gpsimd.memset(mM_t, float(_M))

            q1_t = cpool.tile([128, NG * NS * BS], dt.bfloat16)
            mbuf = spool.tile([128, NG * NS], dt.float32)
            m3buf = spool.tile([128, NG * NS], dt.float32)

            def stage_b_mm(g, s):
                p = ps1.tile([128, BS], dt.float32, tag="ps1", name=f"p1_{g}_{s}")
                nc.tensor.matmul(p, w1_t[0:114, g * 128:(g + 1) * 128],
                                 qx[0:114, s * BS:(s + 1) * BS],
                                 start=True, stop=True)
                return p

            def conv_mm(g, s, bias_rhs=None):
                p3 = ps3.tile([128, BS], dt.float32, tag="ps3",
                              name=f"p3_{g}_{s}")
                dmin = max(-2, -s)
                dmax = min(2, (NS - 1) - s)
                for d in range(dmin, dmax + 1):
                    col0 = ((d + 2) * NG + g) * 128
                    nc.tensor.matmul(
                        p3, wc_t[:, col0:col0 + 128],
                        q1_t[:, (g * NS + s + d) * BS:(g * NS + s + d + 1) * BS],
                        start=(d == dmin), stop=(d == dmax and bias_rhs is None))
                if bias_rhs is not None:
                    nc.tensor.matmul(p3, bcr_t[0:1, g * 128:(g + 1) * 128],
                                     bias_rhs, start=False, stop=True)
                return p3

            # ---------------- PH1: abs-max of stage-B psum -----------------
            for g in range(NG):
                for s in range(NS):
                    p = stage_b_mm(g, s)
                    nc.vector.tensor_reduce(
                        mbuf[:, g * NS + s: g * NS + s + 1], p, axis=AXL.X,
                        op=ALU.max, apply_absolute_value=True)

            mred = spool.tile([128, 1], dt.float32)
            nc.vector.tensor_reduce(mred, mbuf, axis=AXL.X, op=ALU.max)
            m1s = spool.tile([1, 8], dt.float32)
            nc.gpsimd.memset(m1s, 0.0)
            nc.gpsimd.tensor_reduce(m1s[0:1, 0:1], mred, axis=AXL.C, op=ALU.max)

            ar_in1 = dpool.tile([1, 8], dt.float32)
            ar_out1 = dpool.tile([1, 8], dt.float32, addr_space="Shared")
            nc.sync.dma_start(ar_in1, m1s)
            nc.gpsimd.collective_compute(
                "AllReduce", ALU.max, ins=[ar_in1.opt()], outs=[ar_out1.opt()],
                replica_groups=rg)
            m1g = spool.tile([1, 8], dt.float32)
            nc.sync.dma_start(m1g, ar_out1[:])

            # broadcast global max to [128,1] via ones-lhsT matmul
            pb = psb.tile([128, 1], dt.float32, tag="pb", name="pb1")
            nc.tensor.matmul(pb, ones_r, m1g[0:1, 0:1], start=True, stop=True)
            m1t = spool.tile([128, 1], dt.float32)
            nc.scalar.activation(m1t, pb, AFT.Copy)

            # scalar chain 1 (m1t = max|raw+b1/k1| -> s1 = max(m*k1/127,1e-8))
            s1_t = spool.tile([128, 1], dt.float32)
            nc.vector.tensor_mul(s1_t, m1t, k1d127_c)
            nc.vector.tensor_scalar(s1_t, s1_t, float(1e-8), None, ALU.max)
            inv_s1 = spool.tile([128, 1], dt.float32)
            nc.vector.reciprocal(inv_s1, s1_t)
            a1_t = spool.tile([128, 1], dt.float32)
            nc.vector.tensor_mul(a1_t, inv_s1, k1_c)
            th1 = spool.tile([128, 1], dt.float32)
            nc.scalar.activation(th1, s1_t, AFT.Tanh, scale=127.0)
            s2_t = spool.tile([128, 1], dt.float32)
            nc.vector.tensor_scalar(s2_t, th1, float(1.0 / 127.0), float(1e-8),
                                    ALU.mult, ALU.max)
            inv_s2 = spool.tile([128, 1], dt.float32)
            nc.vector.reciprocal(inv_s2, s2_t)
            k3_t = spool.tile([128, 1], dt.float32)
            nc.vector.tensor_mul(k3_t, s2_t, sWc_c)
            inv_k3 = spool.tile([128, 1], dt.float32)
            nc.vector.reciprocal(inv_k3, k3_t)
            # device row [1, BS] of 1/k3 for the conv bias matmul (bf16)
            rk3_f = spool.tile([1, BS], dt.float32)
            nc.vector.scalar_tensor_tensor(rk3_f, ones_b, inv_k3[0:1, 0:1],
                                           ones_b, ALU.mult, ALU.mult)
            rk3 = spool.tile([1, BS], dt.bfloat16)
            nc.vector.tensor_copy(rk3, rk3_f)

            def quant_chain(p, a_ap, bias_ap, sc_ap, invn_ap, qdst, nm):
                """qdst (bf16 ints) = rne(tanh(sc*rne(p*a + bias)) * invn).

                bias_ap may be None when the bias is already inside p (then
                the +M is fused into the ScalarE affine drain)."""
                w = wpool.tile([128, BS], dt.float32, tag="ew", name=f"w{nm}")
                if bias_ap is None:
                    nc.scalar.activation(w, p, AFT.Identity, bias=mM_t,
                                         scale=a_ap)
                    ql = wpool.tile([128, BS], dt.bfloat16, tag="eql",
                                    name=f"ql{nm}")
                    nc.gpsimd.tensor_scalar(ql, w, float(-_M), None, ALU.add)
                else:
                    nc.scalar.activation(w, p, AFT.Identity, bias=bias_ap,
                                         scale=a_ap)
                    ql = wpool.tile([128, BS], dt.bfloat16, tag="eql",
                                    name=f"ql{nm}")
                    nc.vector.tensor_scalar(ql, w, float(_M), float(-_M),
                                            ALU.add, ALU.add)
                t = wpool.tile([128, BS], dt.float32, tag="et", name=f"t{nm}")
                nc.scalar.activation(t, ql, AFT.Tanh, scale=sc_ap)
                v = wpool.tile([128, BS], dt.float32, tag="ev", name=f"v{nm}")
                nc.vector.tensor_scalar(v, t, invn_ap, float(_M),
                                        ALU.mult, ALU.add)
                nc.gpsimd.tensor_scalar(qdst, v, float(-_M), None, ALU.add)

            # ---------------- PH2: q1, conv(+bias), abs-max ----------------
            for g in range(NG):
                for s in range(NS):
                    p = stage_b_mm(g, s)
                    quant_chain(p, a1_t, None, s1_t, inv_s2,
                                q1_t[:, (g * NS + s) * BS:(g * NS + s + 1) * BS],
                                f"b{g}_{s}")
            for g in range(NG):
                for s in range(NS):
                    p3 = conv_mm(g, s, bias_rhs=rk3)
                    nc.vector.tensor_reduce(
                        m3buf[:, g * NS + s: g * NS + s + 1], p3, axis=AXL.X,
                        op=ALU.max, apply_absolute_value=True)

            m3red = spool.tile([128, 1], dt.float32)
            nc.vector.tensor_reduce(m3red, m3buf, axis=AXL.X, op=ALU.max)
            m3s = spool.tile([1, 8], dt.float32)
            nc.gpsimd.memset(m3s, 0.0)
            nc.gpsimd.tensor_reduce(m3s[0:1, 0:1], m3red, axis=AXL.C, op=ALU.max)

            ar_in2 = dpool.tile([1, 8], dt.float32)
            ar_out2 = dpool.tile([1, 8], dt.float32, addr_space="Shared")
            nc.sync.dma_start(ar_in2, m3s)
            nc.gpsimd.collective_compute(
                "AllReduce", ALU.max, ins=[ar_in2.opt()], outs=[ar_out2.opt()],
                replica_groups=rg)
            m3g = spool.tile([1, 8], dt.float32)
            nc.sync.dma_start(m3g, ar_out2[:])
            pb3 = psb.tile([128, 1], dt.float32, tag="pb", name="pb3")
            nc.tensor.matmul(pb3, ones_r, m3g[0:1, 0:1], start=True, stop=True)
            m3t = spool.tile([128, 1], dt.float32)
            nc.scalar.activation(m3t, pb3, AFT.Copy)

            # scalar chain 2: m3 = max|raw3+bc/k3| -> s3 = max(m3*k3/127,1e-8)
            s3_t = spool.tile([128, 1], dt.float32)
            nc.vector.tensor_mul(s3_t, m3t, k3_t)
            nc.vector.tensor_scalar(s3_t, s3_t, float(1.0 / 127.0), float(1e-8),
                                    ALU.mult, ALU.max)
            inv_s3 = spool.tile([128, 1], dt.float32)
            nc.vector.reciprocal(inv_s3, s3_t)
            a3_t = spool.tile([128, 1], dt.float32)
            nc.vector.tensor_mul(a3_t, k3_t, inv_s3)
            th3 = spool.tile([128, 1], dt.float32)
            nc.scalar.activation(th3, s3_t, AFT.Tanh, scale=127.0)
            s4_t = spool.tile([128, 1], dt.float32)
            nc.vector.tensor_scalar(s4_t, th3, float(1.0 / 127.0), float(1e-8),
                                    ALU.mult, ALU.max)
            inv_s4 = spool.tile([128, 1], dt.float32)
            nc.vector.reciprocal(inv_s4, s4_t)
            k5_t = spool.tile([128, 1], dt.float32)
            nc.vector.tensor_mul(k5_t, s4_t, sWf_c)
            bcs3 = spool.tile([128, NG], dt.float32)
            for g in range(NG):
                nc.vector.tensor_mul(bcs3[:, g:g + 1], bc_t[:, g:g + 1], inv_s3)

            # ---------------- PH3: conv recompute, q2, final ---------------
            pf = psf.tile([10, BS], dt.float32)
            n_acc = NG * NS
            idx = 0
            for g in range(NG):
                for s in range(NS):
                    p3 = conv_mm(g, s)
                    q2 = wpool.tile([128, BS], dt.bfloat16, tag="q2",
                                    name=f"q2_{g}_{s}")
                    quant_chain(p3, a3_t, bcs3[:, g:g + 1], s3_t, inv_s4, q2,
                                f"d{g}_{s}")
                    col0 = (s * NG + g) * 10
                    nc.tensor.matmul(pf, wf_t[:, col0:col0 + 10], q2,
                                     start=(idx == 0), stop=(idx == n_acc - 1),
                                     skip_group_check=True)
                    idx += 1

            lg_sb = wpool.tile([10, BS], dt.float32, tag="lg")
            nc.vector.tensor_scalar(lg_sb, pf, k5_t[0:10, 0:1],
                                    bf_c[0:10, 0:1], ALU.mult, ALU.add)
            lg_d = dpool.tile([10, BS], dt.float32)
            nc.sync.dma_start(lg_d, lg_sb)
            lg_all = dpool.tile([N_CORES * 10, BS], dt.float32,
                                addr_space="Shared")
            nc.gpsimd.collective_compute(
                "AllGather", ALU.bypass, ins=[lg_d.opt()], outs=[lg_all.opt()],
                replica_groups=rg)
            nc.sync.dma_start(out_d[:], lg_all[:])

    nc.compile()
    return nc


def _make_runner(nc):
    """Persistent jitted SPMD executable over 8 cores.

    Mirrors concourse.bass2jax.run_bass_via_pjrt's multi-core path, but
    the jit (and hence the traced/lowered/compiled executable) is built
    once at import and reused on every kernel() call.
    """
    bass2jax.install_neuronx_cc_hook()
    partition_name = (nc.partition_id_tensor.name
                      if nc.partition_id_tensor else None)

    in_names, out_names, out_avals = [], [], []
    for alloc in nc.m.functions[0].allocations:
        if not isinstance(alloc, mybir.MemoryLocationSet):
            continue
        name = alloc.memorylocations[0].name
        if alloc.kind == "ExternalInput":
            if name != partition_name:
                in_names.append(name)
        elif alloc.kind == "ExternalOutput":
            shape = tuple(alloc.tensor_shape)
            dtype = mybir.dt.np(alloc.dtype)
            out_names.append(name)
            out_avals.append(jax.core.ShapedArray(shape, dtype))
    n_params = len(in_names)
    # The kernel writes every element of the output, so no donated
    # pre-zeroed output operands are needed (upstream run_bass_via_pjrt
    # threads them only for kernels with partially-written outputs).
    if partition_name is not None:
        in_names.append(partition_name)

    def _body(*args):
        operands = list(args)
        if partition_name is not None:
            operands.append(bass2jax.partition_id_tensor())
        outs = bass2jax._bass_exec_p.bind(
            *operands,
            out_avals=tuple(out_avals),
            in_names=tuple(in_names),
            out_names=tuple(out_names),
            lowering_input_output_aliases=(),
            sim_require_finite=True,
            sim_require_nnan=True,
            nc=nc,
        )
        return tuple(outs)

    devices = jax.devices()[:N_CORES]
    mesh = Mesh(np.asarray(devices), ("core",))
    in_specs = (PartitionSpec("core"),) * n_params
    # the kernel all-gathers logits, so every core returns the identical
    # full [8*10, BS] tensor -> replicated out_spec, single-shard fetch.
    out_specs = (PartitionSpec(),) * len(out_names)
    sharded = jax.jit(
        shard_map(_body, mesh=mesh, in_specs=in_specs, out_specs=out_specs,
                  check_rep=False),
        keep_unused=True,
    )
    return sharded


_NC = _build_nc()
_SHARDED = _make_runner(_NC)


def _run(inp_concat):
    """Execute the persistent jitted kernel; returns [N_CORES, 10, BS]."""
    out = _SHARDED(inp_concat)[0]
    res = np.asarray(out)
    return res.reshape(N_CORES, 10, BS)


# warm everything once at import: NEFF compile, XLA executable, axon path
_run(np.zeros((N_CORES * 128, IC), BF16))


def _pack_weights(W1, b1, Wc, bc, Wf, bf, k1):
    """Quantize weights and pack the [128, WW] device weight matrix."""
    qW1, _ = _quant_weight(W1, 3)             # [384, 28]
    qWc, sWc = _quant_weight(Wc, 3)           # [384, 1, 15]
    qWf, sWf = _quant_weight(Wf, 3)           # [10, 28*384]

    # conv block-Toeplitz [128, 5*12*128]:
    #   W_{d,g}[pidx(li,c), pidx(lo,c)] = qWc[c, li - lo + 4d + 7]
    wcb = np.zeros((4, 32, 5, NG, 4, 32), F32)
    qc = qWc[:, 0, :].reshape(NG, 32, KK)     # [g, c, k]
    ci = np.arange(32)
    for dd in range(5):
        for li in range(4):
            for lo in range(4):
                k = li - lo + 4 * (dd - 2) + PAD
                if 0 <= k < KK:
                    wcb[li, ci, dd, :, lo, ci] = qc[:, :, k].T
    # final lhsT [128, 7*12*10]: row pidx(lp,c) of (s,g)-chunk, col j
    wfq = qWf.reshape(10, NS, 4, NG, 32)      # [j, s, lp, g, c]
    wfb = np.ascontiguousarray(wfq.transpose(2, 4, 1, 3, 0))  # [lp,c,s,g,j]
    # stage-B block [rows 0..111], b1/k1 hi/lo rows 112/113, bcr row 114
    w1b = np.zeros((128, NG * 128), F32)
    w1r = w1b[0:112].reshape(4, 28, NG, 4, 32)
    q1g = qW1.reshape(NG, 32, L).transpose(2, 0, 1)    # [f, g, c]
    for lp in range(4):
        w1r[lp, :, :, lp, :] = q1g
    r = (b1 / F32(k1)).astype(F32)                      # [384]
    hi = r.astype(BF16).astype(F32)
    lo = (r - hi).astype(F32)
    w1b[112].reshape(NG, 4, 32)[:] = hi.reshape(NG, 1, 32)
    w1b[113].reshape(NG, 4, 32)[:] = lo.reshape(NG, 1, 32)
    w1b[114].reshape(NG, 4, 32)[:] = bc.reshape(NG, 1, 32)
    wg = np.concatenate(
        [wcb.reshape(128, 5 * NG * 128), wfb.reshape(128, NS * NG * 10), w1b],
        axis=1)
    return wg, sWc, sWf


def kernel(image, W1, b1, Wc, bc, Wf, bf):
    image = np.asarray(image, F32)
    W1 = np.asarray(W1, F32)
    b1 = np.asarray(b1, F32)
    Wc = np.asarray(Wc, F32)
    bc = np.asarray(bc, F32)
    Wf = np.asarray(Wf, F32)
    bf = np.asarray(bf, F32)

    _qW1, sW1 = _quant_weight(W1, 3)
    s0 = _scale(np.abs(image).max(), 8)      # host: exact global input scale
    k1 = float(s0 * sW1)

    wg, sWc, sWf = _pack_weights(W1, b1, Wc, bc, Wf, bf, k1)

    # host input layout: [b, l, f] -> per core [lp, f, lg, b] = [112, 7*512]
    img = image.reshape(N_CORES, BS, NS, 4, L)      # [core, b, lg, lp, f]
    img = img.transpose(0, 3, 4, 2, 1)               # [core, lp, f, lg, b]
    qimg = _rne(img.reshape(N_CORES, 112, QC) / F32(s0))

    # scalar block: fp32 values as bf16 hi/lo pairs, one column each
    sc = np.zeros((128, NSC), F32)
    sc[:, 0] = F32(k1)
    sc[:, 1] = F32(k1 / 127.0)
    sc[:, 2] = F32(sWc)
    sc[:, 3] = F32(sWf)
    # bcp: per-partition bc columns, one per channel-group g
    sc[:, 4:16] = np.broadcast_to(
        bc.reshape(NG, 1, 32).transpose(1, 2, 0), (4, 32, NG)
    ).reshape(128, NG)
    sc[0:10, 16] = bf
    schi = sc.astype(BF16).astype(F32)
    sclo = sc - schi

    inp = np.zeros((N_CORES, 128, IC), BF16)
    qbytes = inp[:, 0:114, 0:QCB].view(np.int8)      # [core, 114, QC]
    qbytes[:, 0:112, :] = qimg.astype(np.int8)
    qbytes[:, 112:114, :] = 1                        # stage-B bias ones rows
    inp[:, :, WCH0:SC0] = (
        wg.astype(BF16).reshape(N_CORES, 128, WCHW))
    inp[:, :, SC0:SC0 + NSC] = schi.astype(BF16)
    inp[:, :, SC0 + NSC:IC] = sclo.astype(BF16)

    res = _run(inp.reshape(N_CORES * 128, IC))

    # gather + host-side exact final fake-quant (s5 global)
    logits = res.transpose(0, 2, 1).reshape(BATCH, 10)
    s5 = _scale(np.abs(logits).max(), 8)
    out = (_rne(logits / s5) * s5).astype(F32)

    class _R:
        exec_time_ns = None
    kernel.last_results = _R()
    return out
